# revision 6
# baseline (speedup 1.0000x reference)
"""GCC-PHAT Trainium2 kernel (v3: radix-2 DIT forward, fp16 datapath).

Pipeline (per core, batch-sharded B=16 -> 2 per core):
  1. Host permutes samples even|odd: xT[b,m,j,t], j<512 = x[2j], j>=512 =
     x[2j+1]. Forward = two 512-point real DFTs (E over even, O over odd)
     sharing one F512 [512x512] fp16 stationary matrix; fp32 PSUM accum.
     F512 cols (4 chunks of 128): Ea[0..127], Ea[128..255],
     [Ea256, Eb 1..127], Eb[128..255]  (Eb = -sin coeffs).
  2. Radix-2 butterfly on DVE (tensor_scalar 4x + tensor_tensor 2x):
     u = c*oa + s*ob, v = c*ob - s*oa  (W^k = c - i s),
     X_lo = (ea+u, eb+v)  bins k / 128+k,
     X_hi = (ea-u, v-eb)  bins 512-k / 384-k (conjugate-reflected; the
     reversed bin order is baked into the inverse G rows).
     p0 lanes give bins 0 and 512 free; bin 256 patched from preserved
     eb_lo/ob_lo (= Ea256/Oa256).
  3. PHAT normalize per mic, all 8 mics per op ([128, 2000] tiles):
     squares on ACT, sum + y = X*w on DVE, w = Abs_reciprocal_sqrt(16 s)
     on ACT. y-chunk p0 specials: yc0 = (sign X0, sign X512).
  4. Pair products (28 pairs diag-major, 500-col blocks): 13 planes
     (yc0: aa/bb/ab/ba, yc1..3: Karatsuba k1/k2/k3), split DVE/Pool.
  5. Truncated inverse DFT: G [128x64] stationary, 13 accumulating
     matmuls per block into PSUM [64,500]; ACT copy -> DMA out
     [b, lag, diag-pair, t]; host unscrambles to [b, p, t, lag].
"""

import os
from contextlib import ExitStack

import numpy as np

import concourse.bass as bass
import concourse.bacc as bacc
import concourse.mybir as mybir
import concourse.tile as tile
from concourse.bass import ds, ts
from concourse.bass_utils import run_bass_kernel_spmd

B, M, T, L = 16, 8, 250, 1024
NCORES = 8
NB = B // NCORES
NPAIRS = (M * (M - 1)) // 2   # 28
NL = 64
F32 = mybir.dt.float32
FP16 = mybir.dt.float16


def _build_F512() -> np.ndarray:
    j = np.arange(512, dtype=np.float64)[:, None]
    F = np.zeros((512, 512))
    k_lo = np.arange(0, 128, dtype=np.float64)[None, :]
    k_hi = np.arange(128, 256, dtype=np.float64)[None, :]
    F[:, 0:128] = np.cos(2 * np.pi * j * k_lo / 512)
    F[:, 128:256] = np.cos(2 * np.pi * j * k_hi / 512)
    F[:, 256] = np.cos(np.pi * j[:, 0])
    F[:, 257:384] = -np.sin(2 * np.pi * j * k_lo[:, 1:] / 512)
    F[:, 384:512] = -np.sin(2 * np.pi * j * k_hi / 512)
    return F.astype(np.float16)


def _build_G13() -> np.ndarray:
    """[13, 128, NL]: planes 0..3 yc0 aa/bb/ab/ba; 4..6 yc1 k1/k2/k3;
    7..9 yc2; 10..12 yc3. k1=(a1+b1)a2 G=cos-sin; k2=a1(a2+b2) G=sin;
    k3=b1(a2-b2) G=-cos. Product scale 1/16 (y=unit/4) -> 16x in G,
    except yc0 p0 sign slots (scale 1)."""
    G = np.zeros((13, 128, NL))
    nj = (np.arange(NL) - 32).astype(np.float64)

    def cv(f):
        w = 1.0 if f == 512 else 2.0
        return 16.0 * w * np.cos(2 * np.pi * f * nj / L) / L

    def sv(f):
        w = 1.0 if f == 512 else 2.0
        return 16.0 * w * np.sin(2 * np.pi * f * nj / L) / L

    for p in range(1, 128):
        G[0, p] = cv(p)
        G[1, p] = cv(p)
        G[2, p] = sv(p)
        G[3, p] = -sv(p)
    G[0, 0] = 1.0 / L                   # bin 0 (sign products)
    G[1, 0] = np.cos(np.pi * nj) / L    # bin 512 (sign products)
    for p in range(128):
        for base, f in ((4, 128 + p), (7, 256 if p == 0 else 512 - p),
                        (10, 384 - p)):
            G[base + 0, p] = cv(f) - sv(f)
            G[base + 1, p] = sv(f)
            G[base + 2, p] = -cv(f)
    return G.astype(np.float16)


def _build_tw() -> np.ndarray:
    """[128, 4] f32: cols cA, sA, cB, sB (theta = 2 pi k / 1024)."""
    k = np.arange(128, dtype=np.float64)
    tw = np.stack([np.cos(2 * np.pi * k / L), np.sin(2 * np.pi * k / L),
                   np.cos(2 * np.pi * (128 + k) / L),
                   np.sin(2 * np.pi * (128 + k) / L)], axis=1)
    return tw.astype(np.float32)


def build_bass() -> bass.Bass:
    nc = bacc.Bacc("TRN2", target_bir_lowering=False, debug=False)
    xT = nc.dram_tensor("xT", [NB, M, L, T], FP16, kind="ExternalInput")
    out = nc.dram_tensor("out", [NB, NL, NPAIRS, T], F32, kind="ExternalOutput")
    Fh = nc.inline_tensor(_build_F512(), name="Fmat")
    Gh = nc.inline_tensor(np.ascontiguousarray(_build_G13()), name="Gmat")
    Th = nc.inline_tensor(_build_tw(), name="Twid")

    mul = mybir.AluOpType.mult

    with tile.TileContext(nc) as tc, ExitStack() as ctx:
        consts = ctx.enter_context(tc.tile_pool(name="consts", bufs=1))
        xt_pool = ctx.enter_context(tc.tile_pool(name="xt", bufs=5))
        eo1_pool = ctx.enter_context(tc.tile_pool(name="eo1", bufs=1))
        eo2_pool = ctx.enter_context(tc.tile_pool(name="eo2", bufs=2))
        tmp_pool = ctx.enter_context(tc.tile_pool(name="tmp", bufs=1))
        sq_pool = ctx.enter_context(tc.tile_pool(name="sq", bufs=1))
        ysd_pool = ctx.enter_context(tc.tile_pool(name="ysd", bufs=1))
        r_pool = ctx.enter_context(tc.tile_pool(name="r", bufs=2))
        o_pool = ctx.enter_context(tc.tile_pool(name="o", bufs=3))
        fwd_psum = ctx.enter_context(tc.tile_pool(name="fps", bufs=6, space="PSUM"))
        inv_psum = ctx.enter_context(tc.tile_pool(name="ips", bufs=2, space="PSUM"))

        f_sb = consts.tile([128, 4, 512], FP16)
        nc.sync.dma_start(f_sb[:], Fh[:].rearrange("(k p) c -> p k c", p=128))
        g_sb = consts.tile([128, 13, NL], FP16)
        nc.sync.dma_start(g_sb[:], Gh[:].rearrange("i p j -> p i j"))
        tw_sb = consts.tile([128, 4], F32)
        nc.sync.dma_start(tw_sb[:], Th[:])

        for b in range(NB):
            # ---- forward: E (even, k-chunks 0..3) and O (odd, 4..7) ----
            xts = []
            for mg in range(4):
                xt_sb = xt_pool.tile([128, 8, 2, T], FP16, tag="xt")
                for mi in range(2):
                    nc.sync.dma_start(
                        xt_sb[:, :, mi],
                        xT[b, 2 * mg + mi].rearrange("(k p) t -> p k t", p=128),
                    )
                xts.append(xt_sb)

            # ea_lo/ea_hi/eb_hi live through the pair stage (become y
            # chunks) -> double-buffered; the rest die after butterflies.
            eo_names = ["ea_lo", "ea_hi", "ebl", "eb_hi",
                        "oa_lo", "oa_hi", "obl", "ob_hi"]
            eo = {}
            for nm in eo_names:
                pool = eo2_pool if nm in ("ea_lo", "ea_hi", "eb_hi") else eo1_pool
                eo[nm] = pool.tile([128, 4, 2, T], FP16, tag=nm, name=f"{nm}{b}")
            for c in range(4):          # F col chunk
                for half in range(2):   # 0 = even/E, 1 = odd/O
                    dest = eo[eo_names[4 * half + c]]
                    for mg in range(4):
                        ps = fwd_psum.tile([128, 2, T], F32, tag="fp")
                        for kc in range(4):
                            nc.tensor.matmul(
                                ps[:],
                                f_sb[:, kc, ts(c, 128)],
                                xts[mg][:, 4 * half + kc],
                                start=(kc == 0), stop=(kc == 3),
                            )
                        nc.scalar.copy(dest[:, mg], ps[:])

            # ---- butterflies (DVE), [128, (4,2,250)] = 2000-col ops ----
            def flat(tl):
                return tl[:].rearrange("p a b t -> p (a b t)")

            ch = {}  # chunk -> (ya_tile, yb_tile) after in-place normalize
            for grp, (ea_n, eb_n, oa_n, ob_n, ci_lo, ci_hi) in enumerate((
                    ("ea_lo", "ebl", "oa_lo", "obl", 0, 2),
                    ("ea_hi", "eb_hi", "oa_hi", "ob_hi", 1, 3))):
                cc = tw_sb[:, 2 * grp:2 * grp + 1]
                ss = tw_sb[:, 2 * grp + 1:2 * grp + 2]
                ea, eb = flat(eo[ea_n]), flat(eo[eb_n])
                oa, ob = flat(eo[oa_n]), flat(eo[ob_n])
                t1 = flat(tmp_pool.tile([128, 4, 2, T], FP16, tag=f"t1g{grp}", name=f"t1g{grp}b{b}"))
                t2 = flat(tmp_pool.tile([128, 4, 2, T], FP16, tag=f"t2g{grp}", name=f"t2g{grp}b{b}"))
                t3 = flat(tmp_pool.tile([128, 4, 2, T], FP16, tag=f"t3g{grp}", name=f"t3g{grp}b{b}"))
                t4 = flat(tmp_pool.tile([128, 4, 2, T], FP16, tag=f"t4g{grp}", name=f"t4g{grp}b{b}"))
                nc.vector.tensor_scalar_mul(t1, oa, cc)
                nc.vector.tensor_scalar_mul(t2, ob, ss)
                nc.vector.tensor_scalar_mul(t3, oa, ss)
                nc.vector.tensor_scalar_mul(t4, ob, cc)
                nc.vector.tensor_add(t1, t1, t2)   # u
                nc.vector.tensor_sub(t4, t4, t3)   # v
                nc.vector.tensor_add(t2, ea, t1)   # Xa_lo
                nc.vector.tensor_sub(ea, ea, t1)   # Xa_hi (in place)
                if grp == 0:
                    # preserve eb_lo (Ea256 at p0) and ob_lo (Oa256 at p0)
                    nc.vector.tensor_add(t3, eb, t4)   # Xb_lo
                    nc.vector.tensor_sub(t4, t4, eb)   # Xb_hi
                    ch[ci_lo] = (t2, t3)
                    ch[ci_hi] = (ea, t4)
                else:
                    nc.vector.tensor_sub(t3, t4, eb)   # Xb_hi
                    nc.vector.tensor_add(eb, eb, t4)   # Xb_lo (in place)
                    ch[ci_lo] = (t2, eb)
                    ch[ci_hi] = (ea, t3)

            # ---- normalize: y = X / (4|X|), in place over X ----
            def norm_chunk(ci):
                xa, xb = ch[ci]
                sq_a = flat(sq_pool.tile([128, 4, 2, T], FP16, tag="sqa", name=f"sqa{ci}b{b}"))
                sq_b = flat(sq_pool.tile([128, 4, 2, T], FP16, tag="sqb", name=f"sqb{ci}b{b}"))
                w = flat(sq_pool.tile([128, 4, 2, T], FP16, tag="w", name=f"w{ci}b{b}"))
                nc.scalar.square(sq_a, xa)
                nc.scalar.square(sq_b, xb)
                nc.vector.tensor_add(sq_a, sq_a, sq_b)
                nc.scalar.activation(
                    w, sq_a, mybir.ActivationFunctionType.Abs_reciprocal_sqrt,
                    scale=16.0)
                nc.vector.tensor_mul(xa, xa, w)    # ya (in place)
                nc.vector.tensor_mul(xb, xb, w)    # yb (in place)

            ysd = {}
            xa0, xb0 = ch[0]
            xa2, xb2 = ch[2]
            ebl, obl = flat(eo["ebl"]), flat(eo["obl"])
            # yc0 first; its p0 slots become sign(X0), sign(X512).
            norm_chunk(0)
            nc.scalar.sign(xa0[0:1], xa0[0:1])     # sign(X0*w) = sign(X0)
            nc.scalar.sign(xb0[0:1], xa2[0:1])     # Xa_hi[p0] = X512, pre-patch
            # bin-256 patch, then yc2 normalize handles it uniformly
            nc.scalar.copy(xa2[0:1], ebl[0:1])     # Ea256
            nc.scalar.activation(xb2[0:1], obl[0:1],
                                 mybir.ActivationFunctionType.Copy,
                                 scale=-1.0)       # -Oa256
            for ci in (1, 2, 3):
                norm_chunk(ci)
                ys = flat(ysd_pool.tile([128, 4, 2, T], FP16, tag=f"ys{ci}", name=f"ys{ci}b{b}"))
                yd = flat(ysd_pool.tile([128, 4, 2, T], FP16, tag=f"yd{ci}", name=f"yd{ci}b{b}"))
                xa, xb = ch[ci]
                nc.vector.tensor_add(ys, xa, xb)
                nc.vector.tensor_sub(yd, xa, xb)
                ysd[ci] = (ys, yd)

            # ---- pairs (diag-major) + inverse, 500-col blocks ----
            # plane -> engine: 5 of 13 on Pool (gpsimd), rest DVE
            pool_planes = {1, 3, 6, 9, 12}

            kb = 0
            for d in range(1, M):
                lanes = M - d
                for l0 in range(0, lanes, 2):
                    lc = min(2, lanes - l0)
                    rows = lc * T
                    s1 = ds(l0 * T, rows)
                    s2 = ds((l0 + d) * T, rows)
                    r_sb = r_pool.tile([128, 13, 2 * T], FP16, tag="r")

                    def prod(idx, a1, a2):
                        eng = nc.gpsimd if idx in pool_planes else nc.vector
                        eng.tensor_mul(r_sb[:, idx, :rows], a1[:, s1], a2[:, s2])

                    ya0, yb0 = ch[0]
                    prod(0, ya0, ya0)
                    prod(1, yb0, yb0)
                    prod(2, ya0, yb0)
                    prod(3, yb0, ya0)
                    for ci, base in ((1, 4), (2, 7), (3, 10)):
                        ya_, yb_ = ch[ci]
                        ys_, yd_ = ysd[ci]
                        prod(base + 0, ys_, ya_)
                        prod(base + 1, ya_, ys_)
                        prod(base + 2, yb_, yd_)

                    ps_o = inv_psum.tile([64, 2 * T], F32, tag="ops")
                    for idx in range(13):
                        nc.tensor.matmul(
                            ps_o[:, :rows],
                            g_sb[:, idx],
                            r_sb[:, idx, :rows],
                            start=(idx == 0), stop=(idx == 12),
                        )
                    o_sb = o_pool.tile([64, 2, T], F32, tag="osb")
                    nc.scalar.copy(
                        o_sb[:, :lc],
                        ps_o[:, :rows].rearrange("p (l t) -> p l t", t=T),
                    )
                    nc.sync.dma_start(
                        out[b, :, ds(kb + l0, lc)],
                        o_sb[:, :lc],
                    )
                kb += lanes
    nc.compile()
    return nc


_NC_CACHE = None


def kernel(x: np.ndarray) -> np.ndarray:
    global _NC_CACHE
    x = np.asarray(x, dtype=np.float32)
    assert x.shape == (B, M, T, L)
    perm = np.concatenate([np.arange(0, L, 2), np.arange(1, L, 2)])
    xT = np.ascontiguousarray(
        x.transpose(0, 1, 3, 2)[:, :, perm]).astype(np.float16)
    if _NC_CACHE is None:
        _NC_CACHE = build_bass()
    nc = _NC_CACHE
    in_maps = [{"xT": xT[c * NB:(c + 1) * NB]} for c in range(NCORES)]
    trace = bool(int(os.environ.get("GCC_TRACE", "0")))
    res = run_bass_kernel_spmd(nc, in_maps, core_ids=list(range(NCORES)),
                               trace=trace)
    if trace and res.exec_time_ns is not None:
        print(f"HW exec time: {res.exec_time_ns} ns")
        if res.instructions_and_trace is not None:
            print("trace:", res.instructions_and_trace[1])
    out = np.concatenate([r["out"] for r in res.results], axis=0)  # [B,NL,28diag,T]
    plist = [m * (2 * M - m - 1) // 2 + (m + d - m - 1)
             for d in range(1, M) for m in range(M - d)]
    final = np.empty((B, NPAIRS, T, NL), dtype=np.float32)
    final[:, plist] = out.transpose(0, 2, 3, 1)
    return final


# revision 17
# speedup vs baseline: 1.0181x; 1.0181x over previous
"""GCC-PHAT Trainium2 kernel (v3: radix-2 DIT forward, fp16 datapath).

Pipeline (per core, batch-sharded B=16 -> 2 per core):
  1. Host permutes samples even|odd: xT[b,m,j,t], j<512 = x[2j], j>=512 =
     x[2j+1]. Forward = two 512-point real DFTs (E over even, O over odd)
     sharing one F512 [512x512] fp16 stationary matrix; fp32 PSUM accum.
     F512 cols (4 chunks of 128): Ea[0..127], Ea[128..255],
     [Ea256, Eb 1..127], Eb[128..255]  (Eb = -sin coeffs).
  2. Radix-2 butterfly on DVE (tensor_scalar 4x + tensor_tensor 2x):
     u = c*oa + s*ob, v = c*ob - s*oa  (W^k = c - i s),
     X_lo = (ea+u, eb+v)  bins k / 128+k,
     X_hi = (ea-u, v-eb)  bins 512-k / 384-k (conjugate-reflected; the
     reversed bin order is baked into the inverse G rows).
     p0 lanes give bins 0 and 512 free; bin 256 patched from preserved
     eb_lo/ob_lo (= Ea256/Oa256).
  3. PHAT normalize per mic, all 8 mics per op ([128, 2000] tiles):
     squares on ACT, sum + y = X*w on DVE, w = Abs_reciprocal_sqrt(16 s)
     on ACT. y-chunk p0 specials: yc0 = (sign X0, sign X512).
  4. Pair products (28 pairs diag-major, 500-col blocks): 13 planes
     (yc0: aa/bb/ab/ba, yc1..3: Karatsuba k1/k2/k3), split DVE/Pool.
  5. Truncated inverse DFT: G [128x64] stationary, 13 accumulating
     matmuls per block into PSUM [64,500]; ACT copy -> DMA out
     [b, lag, diag-pair, t]; host unscrambles to [b, p, t, lag].
"""

import os
from contextlib import ExitStack

import numpy as np

import concourse.bass as bass
import concourse.bacc as bacc
import concourse.mybir as mybir
import concourse.tile as tile
from concourse.bass import ds, ts
from concourse.bass_utils import run_bass_kernel_spmd

B, M, T, L = 16, 8, 250, 1024
NCORES = 8
NB = B // NCORES
NPAIRS = (M * (M - 1)) // 2   # 28
NL = 64
F32 = mybir.dt.float32
FP16 = mybir.dt.float16


def _build_FEO() -> np.ndarray:
    """[2, 512, 512]: h=0 even-stream (n=2j), h=1 odd-stream (n=2j+1) DFT
    matrices with the radix-2 twiddle absorbed into the odd columns.
    Cols: 0..255 cos(2 pi n k / 1024) k=0..255; col 256 = cos(pi j)
    (bin 256 of the stream's own 512-DFT); 257..511 -sin(...) k=1..255."""
    j = np.arange(512, dtype=np.float64)[:, None]
    F = np.zeros((2, 512, 512))
    k_lo = np.arange(0, 128, dtype=np.float64)[None, :]
    k_hi = np.arange(128, 256, dtype=np.float64)[None, :]
    for h in range(2):
        n = 2 * j + h
        F[h, :, 0:128] = np.cos(2 * np.pi * n * k_lo / L)
        F[h, :, 128:256] = np.cos(2 * np.pi * n * k_hi / L)
        F[h, :, 256] = np.cos(np.pi * j[:, 0])
        F[h, :, 257:384] = -np.sin(2 * np.pi * n * k_lo[:, 1:] / L)
        F[h, :, 384:512] = -np.sin(2 * np.pi * n * k_hi / L)
    return F.astype(np.float16)


def _build_G13() -> np.ndarray:
    """[13, 128, NL]: planes 0..3 yc0 aa/bb/ab/ba; 4..6 yc1 k1/k2/k3;
    7..9 yc2; 10..12 yc3. k1=(a1+b1)a2 G=cos-sin; k2=a1(a2+b2) G=sin;
    k3=b1(a2-b2) G=-cos. Product scale 1/16 (y=unit/4) -> 16x in G,
    except yc0 p0 sign slots (scale 1)."""
    G = np.zeros((13, 128, NL))
    nj = (np.arange(NL) - 32).astype(np.float64)

    def cv(f):
        w = 1.0 if f == 512 else 2.0
        return 16.0 * w * np.cos(2 * np.pi * f * nj / L) / L

    def sv(f):
        w = 1.0 if f == 512 else 2.0
        return 16.0 * w * np.sin(2 * np.pi * f * nj / L) / L

    for p in range(1, 128):
        G[0, p] = cv(p)
        G[1, p] = cv(p)
        G[2, p] = sv(p)
        G[3, p] = -sv(p)
    G[0, 0] = 1.0 / L                   # bin 0 (sign products)
    G[1, 0] = np.cos(np.pi * nj) / L    # bin 512 (sign products)
    for p in range(128):
        for base, f in ((4, 128 + p), (7, 256 if p == 0 else 512 - p),
                        (10, 384 - p)):
            G[base + 0, p] = cv(f) - sv(f)
            G[base + 1, p] = sv(f)
            G[base + 2, p] = -cv(f)
    return G.astype(np.float16)


def build_bass() -> bass.Bass:
    nc = bacc.Bacc("TRN2", target_bir_lowering=False, debug=False)
    xT = nc.dram_tensor("xT", [NB, M, L, T], FP16, kind="ExternalInput")
    out = nc.dram_tensor("out", [NB, NL, NPAIRS, T], F32, kind="ExternalOutput")
    Fh = nc.inline_tensor(np.ascontiguousarray(_build_FEO()), name="Fmat")
    Gh = nc.inline_tensor(np.ascontiguousarray(_build_G13()), name="Gmat")

    with tile.TileContext(nc) as tc, ExitStack() as ctx:
        consts = ctx.enter_context(tc.tile_pool(name="consts", bufs=1))
        xt_pool = ctx.enter_context(tc.tile_pool(name="xt", bufs=5))
        eo1_pool = ctx.enter_context(tc.tile_pool(name="eo1", bufs=1))
        eo2_pool = ctx.enter_context(tc.tile_pool(name="eo2", bufs=2))
        tmp_pool = ctx.enter_context(tc.tile_pool(name="tmp", bufs=1))
        sq_pool = ctx.enter_context(tc.tile_pool(name="sq", bufs=1))
        ysd_pool = ctx.enter_context(tc.tile_pool(name="ysd", bufs=1))
        r_pool = ctx.enter_context(tc.tile_pool(name="r", bufs=3))
        o_pool = ctx.enter_context(tc.tile_pool(name="o", bufs=2))
        fwd_psum = ctx.enter_context(tc.tile_pool(name="fps", bufs=6, space="PSUM"))
        inv_psum = ctx.enter_context(tc.tile_pool(name="ips", bufs=2, space="PSUM"))

        f_sb = consts.tile([128, 2, 4, 512], FP16)
        nc.sync.dma_start(f_sb[:], Fh[:].rearrange("h (k p) c -> p h k c", p=128))
        g_sb = consts.tile([128, 13, NL], FP16)
        nc.sync.dma_start(g_sb[:], Gh[:].rearrange("i p j -> p i j"))

        for b in range(NB):
            # ---- forward: E (even, k-chunks 0..3) and O (odd, 4..7) ----
            xts = []
            for mg in range(4):
                xt_sb = xt_pool.tile([128, 8, 2, T], FP16, tag="xt")
                for mi in range(2):
                    nc.sync.dma_start(
                        xt_sb[:, :, mi],
                        xT[b, 2 * mg + mi].rearrange("(k p) t -> p k t", p=128),
                    )
                xts.append(xt_sb)

            # ea_lo/ul/ea_hi/eb_hi live through the pair stage (become y
            # chunks) -> double-buffered; the rest die after the butterfly
            # phase (ebl/vl are only read by the bin-256 patch).
            eo_names = ["ea_lo", "ea_hi", "ebl", "eb_hi",
                        "ul", "u_hi", "vl", "v_hi"]
            eo = {}
            for nm in eo_names:
                pool = eo2_pool if nm in ("ea_lo", "ul", "ea_hi", "eb_hi") else eo1_pool
                eo[nm] = pool.tile([128, 4, 2, T], FP16, tag=nm, name=f"{nm}{b}")
            for c in range(4):          # F col chunk
                for half in range(2):   # 0 = even/E, 1 = odd/U,V
                    dest = eo[eo_names[4 * half + c]]
                    for mg in range(4):
                        ps = fwd_psum.tile([128, 2, T], F32, tag="fp")
                        for kc in range(4):
                            nc.tensor.matmul(
                                ps[:],
                                f_sb[:, half, kc, ts(c, 128)],
                                xts[mg][:, 4 * half + kc],
                                start=(kc == 0), stop=(kc == 3),
                            )
                        nc.scalar.copy(dest[:, mg], ps[:])

            # ---- butterflies: 8 DVE adds on [128, (4,2,250)] tiles ----
            def flat(tl):
                return tl[:].rearrange("p a b t -> p (a b t)")

            ea_lo, ea_hi = flat(eo["ea_lo"]), flat(eo["ea_hi"])
            ebl, eb_hi = flat(eo["ebl"]), flat(eo["eb_hi"])
            ul, u_hi = flat(eo["ul"]), flat(eo["u_hi"])
            vl, v_hi = flat(eo["vl"]), flat(eo["v_hi"])
            tA = flat(tmp_pool.tile([128, 4, 2, T], FP16, tag="tA", name=f"tAb{b}"))
            tB = flat(tmp_pool.tile([128, 4, 2, T], FP16, tag="tB", name=f"tBb{b}"))
            tC = flat(tmp_pool.tile([128, 4, 2, T], FP16, tag="tC", name=f"tCb{b}"))
            tD = flat(tmp_pool.tile([128, 4, 2, T], FP16, tag="tD", name=f"tDb{b}"))
            nc.vector.tensor_sub(tA, ea_lo, ul)       # Xa_hi lo-group
            nc.vector.tensor_add(ea_lo, ea_lo, ul)    # Xa_lo (in place)
            nc.vector.tensor_sub(tB, vl, ebl)         # Xb_hi (ebl/vl preserved)
            nc.vector.tensor_add(ul, ebl, vl)         # Xb_lo -> ul
            nc.vector.tensor_sub(tC, ea_hi, u_hi)     # Xa_hi hi-group
            nc.vector.tensor_add(ea_hi, ea_hi, u_hi)  # Xa_lo (in place)
            nc.vector.tensor_sub(tD, v_hi, eb_hi)     # Xb_hi
            nc.vector.tensor_add(eb_hi, eb_hi, v_hi)  # Xb_lo (in place)
            ch = {0: (ea_lo, ul), 1: (ea_hi, eb_hi), 2: (tA, tB), 3: (tC, tD)}

            # ---- normalize: y = X / (4|X|), in place over X ----
            def norm_chunk(ci):
                xa, xb = ch[ci]
                sq_a = flat(sq_pool.tile([128, 4, 2, T], FP16, tag="sqa", name=f"sqa{ci}b{b}"))
                sq_b = flat(sq_pool.tile([128, 4, 2, T], FP16, tag="sqb", name=f"sqb{ci}b{b}"))
                w = flat(sq_pool.tile([128, 4, 2, T], FP16, tag="w", name=f"w{ci}b{b}"))
                nc.scalar.square(sq_a, xa)
                nc.scalar.square(sq_b, xb)
                nc.vector.tensor_add(sq_a, sq_a, sq_b)
                nc.scalar.activation(
                    w, sq_a, mybir.ActivationFunctionType.Abs_reciprocal_sqrt,
                    scale=16.0)
                nc.vector.tensor_mul(xa, xa, w)    # ya (in place)
                nc.vector.tensor_mul(xb, xb, w)    # yb (in place)

            ysd = {}
            xa0, xb0 = ch[0]
            xa2, xb2 = ch[2]
            # yc0 first; its p0 slots become sign(X0), sign(X512).
            norm_chunk(0)
            nc.scalar.sign(xa0[0:1], xa0[0:1])     # sign(X0*w) = sign(X0)
            nc.scalar.sign(xb0[0:1], xa2[0:1])     # Xa_hi[p0] = X512, pre-patch
            # bin-256 patch, then yc2 normalize handles it uniformly
            nc.scalar.copy(xa2[0:1], ebl[0:1])     # Ea256
            nc.scalar.activation(xb2[0:1], vl[0:1],
                                 mybir.ActivationFunctionType.Copy,
                                 scale=-1.0)       # -Oa256
            for ci in (1, 2, 3):
                norm_chunk(ci)
                ys = flat(ysd_pool.tile([128, 4, 2, T], FP16, tag=f"ys{ci}", name=f"ys{ci}b{b}"))
                yd = flat(ysd_pool.tile([128, 4, 2, T], FP16, tag=f"yd{ci}", name=f"yd{ci}b{b}"))
                xa, xb = ch[ci]
                nc.gpsimd.tensor_add(ys, xa, xb)
                nc.gpsimd.tensor_sub(yd, xa, xb)
                ysd[ci] = (ys, yd)

            # ---- pairs (diag-major) + inverse, 500-col blocks ----
            # Whole blocks (13 planes) go to ONE engine: every 5th to Pool,
            # rest to DVE -- concurrent DVE+Pool on the same tiles was
            # measured to ~3.5x-slow the DVE ops (shared SBUF ports).
            # Pool blocks' matmuls are emitted LAG blocks later so the slow
            # producer doesn't head-of-line-block the PE queue.
            blocks = []
            kb = 0
            for d in range(1, M):
                lanes = M - d
                for l0 in range(0, lanes, 2):
                    lc = min(2, lanes - l0)
                    blocks.append((d, l0, lc, kb))
                kb += lanes

            LAG = 2

            def emit_block_matmuls(blk, r_sb):
                d, l0, lc, kb0 = blk
                rows = lc * T
                ps_o = inv_psum.tile([64, 2 * T], F32, tag="ops")
                for idx in range(13):
                    nc.tensor.matmul(
                        ps_o[:, :rows],
                        g_sb[:, idx],
                        r_sb[:, idx, :rows],
                        start=(idx == 0), stop=(idx == 12),
                    )
                o_sb = o_pool.tile([64, 2, T], F32, tag="osb")
                nc.scalar.copy(
                    o_sb[:, :lc],
                    ps_o[:, :rows].rearrange("p (l t) -> p l t", t=T),
                )
                nc.sync.dma_start(out[b, :, ds(kb0 + l0, lc)], o_sb[:, :lc])

            pending = []
            for i, blk in enumerate(blocks):
                d, l0, lc, kb0 = blk
                rows = lc * T
                s1 = ds(l0 * T, rows)
                s2 = ds((l0 + d) * T, rows)
                eng = nc.gpsimd if i % 5 == 2 else nc.vector
                r_sb = r_pool.tile([128, 13, 2 * T], FP16, tag="r")

                def prod(idx, a1, a2):
                    eng.tensor_mul(r_sb[:, idx, :rows], a1[:, s1], a2[:, s2])

                ya0, yb0 = ch[0]
                prod(0, ya0, ya0)
                prod(1, yb0, yb0)
                prod(2, ya0, yb0)
                prod(3, yb0, ya0)
                for ci, base in ((1, 4), (2, 7), (3, 10)):
                    ya_, yb_ = ch[ci]
                    ys_, yd_ = ysd[ci]
                    prod(base + 0, ys_, ya_)
                    prod(base + 1, ya_, ys_)
                    prod(base + 2, yb_, yd_)

                pending.append((blk, r_sb))
                if len(pending) > LAG:
                    emit_block_matmuls(*pending.pop(0))
            for pb in pending:
                emit_block_matmuls(*pb)
    nc.compile()
    return nc


_NC_CACHE = None


def kernel(x: np.ndarray) -> np.ndarray:
    global _NC_CACHE
    x = np.asarray(x, dtype=np.float32)
    assert x.shape == (B, M, T, L)
    perm = np.concatenate([np.arange(0, L, 2), np.arange(1, L, 2)])
    xT = np.ascontiguousarray(
        x.transpose(0, 1, 3, 2)[:, :, perm]).astype(np.float16)
    if _NC_CACHE is None:
        _NC_CACHE = build_bass()
    nc = _NC_CACHE
    in_maps = [{"xT": xT[c * NB:(c + 1) * NB]} for c in range(NCORES)]
    trace = bool(int(os.environ.get("GCC_TRACE", "0")))
    res = run_bass_kernel_spmd(nc, in_maps, core_ids=list(range(NCORES)),
                               trace=trace)
    if trace and res.exec_time_ns is not None:
        print(f"HW exec time: {res.exec_time_ns} ns")
        if res.instructions_and_trace is not None:
            print("trace:", res.instructions_and_trace[1])
    out = np.concatenate([r["out"] for r in res.results], axis=0)  # [B,NL,28diag,T]
    plist = [m * (2 * M - m - 1) // 2 + (m + d - m - 1)
             for d in range(1, M) for m in range(M - d)]
    final = np.empty((B, NPAIRS, T, NL), dtype=np.float32)
    final[:, plist] = out.transpose(0, 2, 3, 1)
    return final


# revision 21
# speedup vs baseline: 1.2942x; 1.2713x over previous
"""GCC-PHAT Trainium2 kernel (v3: radix-2 DIT forward, fp16 datapath).

Pipeline (per core, batch-sharded B=16 -> 2 per core):
  1. Host permutes samples even|odd: xT[b,m,j,t], j<512 = x[2j], j>=512 =
     x[2j+1]. Forward = two 512-point real DFTs (E over even, O over odd)
     sharing one F512 [512x512] fp16 stationary matrix; fp32 PSUM accum.
     F512 cols (4 chunks of 128): Ea[0..127], Ea[128..255],
     [Ea256, Eb 1..127], Eb[128..255]  (Eb = -sin coeffs).
  2. Radix-2 butterfly on DVE (tensor_scalar 4x + tensor_tensor 2x):
     u = c*oa + s*ob, v = c*ob - s*oa  (W^k = c - i s),
     X_lo = (ea+u, eb+v)  bins k / 128+k,
     X_hi = (ea-u, v-eb)  bins 512-k / 384-k (conjugate-reflected; the
     reversed bin order is baked into the inverse G rows).
     p0 lanes give bins 0 and 512 free; bin 256 patched from preserved
     eb_lo/ob_lo (= Ea256/Oa256).
  3. PHAT normalize per mic, all 8 mics per op ([128, 2000] tiles):
     squares on ACT, sum + y = X*w on DVE, w = Abs_reciprocal_sqrt(16 s)
     on ACT. y-chunk p0 specials: yc0 = (sign X0, sign X512).
  4. Pair products (28 pairs diag-major, 500-col blocks): 13 planes
     (yc0: aa/bb/ab/ba, yc1..3: Karatsuba k1/k2/k3), split DVE/Pool.
  5. Truncated inverse DFT: G [128x64] stationary, 13 accumulating
     matmuls per block into PSUM [64,500]; ACT copy -> DMA out
     [b, lag, diag-pair, t]; host unscrambles to [b, p, t, lag].
"""

import os
from contextlib import ExitStack

import numpy as np

import concourse.bass as bass
import concourse.bacc as bacc
import concourse.mybir as mybir
import concourse.tile as tile
from concourse.bass import ds, ts
from concourse.bass_utils import run_bass_kernel_spmd

B, M, T, L = 16, 8, 250, 1024
NCORES = 8
NB = B // NCORES
NPAIRS = (M * (M - 1)) // 2   # 28
NL = 64
F32 = mybir.dt.float32
FP16 = mybir.dt.float16


def _build_FEO() -> np.ndarray:
    """[2, 512, 512]: h=0 even-stream (n=2j), h=1 odd-stream (n=2j+1) DFT
    matrices with the radix-2 twiddle absorbed into the odd columns.
    Cols: 0..255 cos(2 pi n k / 1024) k=0..255; col 256 = cos(pi j)
    (bin 256 of the stream's own 512-DFT); 257..511 -sin(...) k=1..255."""
    j = np.arange(512, dtype=np.float64)[:, None]
    F = np.zeros((2, 512, 512))
    k_lo = np.arange(0, 128, dtype=np.float64)[None, :]
    k_hi = np.arange(128, 256, dtype=np.float64)[None, :]
    for h in range(2):
        n = 2 * j + h
        F[h, :, 0:128] = np.cos(2 * np.pi * n * k_lo / L)
        F[h, :, 128:256] = np.cos(2 * np.pi * n * k_hi / L)
        F[h, :, 256] = np.cos(np.pi * j[:, 0])
        F[h, :, 257:384] = -np.sin(2 * np.pi * n * k_lo[:, 1:] / L)
        F[h, :, 384:512] = -np.sin(2 * np.pi * n * k_hi / L)
    return F.astype(np.float16)


def _build_G13() -> np.ndarray:
    """[13, 128, NL]: planes 0..3 yc0 aa/bb/ab/ba; 4..6 yc1 k1/k2/k3;
    7..9 yc2; 10..12 yc3. k1=(a1+b1)a2 G=cos-sin; k2=a1(a2+b2) G=sin;
    k3=b1(a2-b2) G=-cos. Product scale 1/16 (y=unit/4) -> 16x in G,
    except yc0 p0 sign slots (scale 1)."""
    G = np.zeros((13, 128, NL))
    nj = (np.arange(NL) - 32).astype(np.float64)

    def cv(f):
        w = 1.0 if f == 512 else 2.0
        return 16.0 * w * np.cos(2 * np.pi * f * nj / L) / L

    def sv(f):
        w = 1.0 if f == 512 else 2.0
        return 16.0 * w * np.sin(2 * np.pi * f * nj / L) / L

    for p in range(1, 128):
        G[0, p] = cv(p)
        G[1, p] = cv(p)
        G[2, p] = sv(p)
        G[3, p] = -sv(p)
    G[0, 0] = 1.0 / L                   # bin 0 (sign products)
    G[1, 0] = np.cos(np.pi * nj) / L    # bin 512 (sign products)
    # planes 4..6 = k1 of (yc1, yc2, yc3); 7..9 = k2; 10..12 = k3 --
    # grouped by k so one merged DVE op writes 3 contiguous planes.
    for p in range(128):
        for j, f in ((0, 128 + p), (1, 256 if p == 0 else 512 - p),
                     (2, 384 - p)):
            G[4 + j, p] = cv(f) - sv(f)
            G[7 + j, p] = sv(f)
            G[10 + j, p] = -cv(f)
    return G.astype(np.float16)


def build_bass() -> bass.Bass:
    nc = bacc.Bacc("TRN2", target_bir_lowering=False, debug=False)
    xT = nc.dram_tensor("xT", [NB, M, L, T], FP16, kind="ExternalInput")
    out = nc.dram_tensor("out", [NB, NL, NPAIRS, T], F32, kind="ExternalOutput")
    Fh = nc.inline_tensor(np.ascontiguousarray(_build_FEO()), name="Fmat")
    Gh = nc.inline_tensor(np.ascontiguousarray(_build_G13()), name="Gmat")

    with tile.TileContext(nc) as tc, ExitStack() as ctx:
        consts = ctx.enter_context(tc.tile_pool(name="consts", bufs=1))
        xt_pool = ctx.enter_context(tc.tile_pool(name="xt", bufs=5))
        eo1_pool = ctx.enter_context(tc.tile_pool(name="eo1", bufs=1))
        eo2_pool = ctx.enter_context(tc.tile_pool(name="eo2", bufs=2))
        y_pool = ctx.enter_context(tc.tile_pool(name="y", bufs=1))
        sq_pool = ctx.enter_context(tc.tile_pool(name="sq", bufs=1))
        r_pool = ctx.enter_context(tc.tile_pool(name="r", bufs=2))
        o_pool = ctx.enter_context(tc.tile_pool(name="o", bufs=2))
        fwd_psum = ctx.enter_context(tc.tile_pool(name="fps", bufs=6, space="PSUM"))
        inv_psum = ctx.enter_context(tc.tile_pool(name="ips", bufs=2, space="PSUM"))

        f_sb = consts.tile([128, 2, 4, 512], FP16)
        nc.sync.dma_start(f_sb[:], Fh[:].rearrange("h (k p) c -> p h k c", p=128))
        g_sb = consts.tile([128, 13, NL], FP16)
        nc.sync.dma_start(g_sb[:], Gh[:].rearrange("i p j -> p i j"))

        EO_NAMES = ["ea_lo", "ea_hi", "ebl", "eb_hi", "ul", "u_hi", "vl", "v_hi"]
        # drain order: lo-group inputs (c0, c2) first so lo butterflies start
        # at half-forward
        STEP_ORDER = [(0, 0), (0, 1), (2, 0), (2, 1), (1, 0), (1, 1), (3, 0), (3, 1)]

        def flat(tl):
            return tl[:].rearrange("p a b t -> p (a b t)")

        def make_state(b):
            xts = []
            for mg in range(4):
                xt_sb = xt_pool.tile([128, 8, 2, T], FP16, tag="xt")
                for mi in range(2):
                    nc.sync.dma_start(
                        xt_sb[:, :, mi],
                        xT[b, 2 * mg + mi].rearrange("(k p) t -> p k t", p=128),
                    )
                xts.append(xt_sb)
            eo = {}
            for nm in EO_NAMES:
                # ea_lo/ul become yc0 (live through pair stage) -> 2 bufs
                pool = eo2_pool if nm in ("ea_lo", "ul") else eo1_pool
                eo[nm] = pool.tile([128, 4, 2, T], FP16, tag=nm, name=f"{nm}{b}")
            Ya = y_pool.tile([128, 3, 4, 2, T], FP16, tag="Ya", name=f"Ya{b}")
            Yb = y_pool.tile([128, 3, 4, 2, T], FP16, tag="Yb", name=f"Yb{b}")
            Ys = y_pool.tile([128, 3, 4, 2, T], FP16, tag="Ys", name=f"Ys{b}")
            Yd = y_pool.tile([128, 3, 4, 2, T], FP16, tag="Yd", name=f"Yd{b}")
            return dict(b=b, xts=xts, eo=eo, Y=(Ya, Yb, Ys, Yd))

        def fwd_step(st, step):
            c, half = step
            dest = st["eo"][EO_NAMES[4 * half + c]]
            for mg in range(4):
                ps = fwd_psum.tile([128, 2, T], F32, tag="fp")
                for kc in range(4):
                    nc.tensor.matmul(
                        ps[:],
                        f_sb[:, half, kc, ts(c, 128)],
                        st["xts"][mg][:, 4 * half + kc],
                        start=(kc == 0), stop=(kc == 3),
                    )
                nc.scalar.copy(dest[:, mg], ps[:])

        def butterfly_norm(st):
            b = st["b"]
            eo = st["eo"]
            Ya, Yb, Ys, Yd = st["Y"]
            ea_lo, ea_hi = flat(eo["ea_lo"]), flat(eo["ea_hi"])
            ebl, eb_hi = flat(eo["ebl"]), flat(eo["eb_hi"])
            ul, u_hi = flat(eo["ul"]), flat(eo["u_hi"])
            vl, v_hi = flat(eo["vl"]), flat(eo["v_hi"])
            YaF = Ya[:].rearrange("p c a b t -> p c (a b t)")  # [128,3,2000]
            YbF = Yb[:].rearrange("p c a b t -> p c (a b t)")
            # butterflies (all DVE): yc0 in eo tiles, yc1/2/3 in Y slots 0/1/2
            nc.vector.tensor_sub(YaF[:, 1], ea_lo, ul)     # yc2-a (Xa_hi lo)
            nc.vector.tensor_add(ea_lo, ea_lo, ul)         # yc0-a (in place)
            nc.vector.tensor_sub(YbF[:, 1], vl, ebl)       # yc2-b
            nc.vector.tensor_add(ul, ebl, vl)              # yc0-b -> ul
            nc.vector.tensor_sub(YaF[:, 2], ea_hi, u_hi)   # yc3-a (Xa_hi hi)
            nc.vector.tensor_add(YaF[:, 0], ea_hi, u_hi)   # yc1-a
            nc.vector.tensor_sub(YbF[:, 2], v_hi, eb_hi)   # yc3-b
            nc.vector.tensor_add(YbF[:, 0], eb_hi, v_hi)   # yc1-b

            arsq = mybir.ActivationFunctionType.Abs_reciprocal_sqrt
            sqA = sq_pool.tile([128, 3, 4, 2, T], FP16, tag="sqA", name=f"sqA{b}")
            sqB = sq_pool.tile([128, 3, 4, 2, T], FP16, tag="sqB", name=f"sqB{b}")
            sqAF = sqA[:].rearrange("p c a b t -> p c (a b t)")
            sqBF = sqB[:].rearrange("p c a b t -> p c (a b t)")
            # yc0 normalize (uses slot-0 slices of sq tiles as scratch)
            nc.scalar.square(sqAF[:, 0], ea_lo)
            nc.scalar.square(sqBF[:, 0], ul)
            nc.vector.tensor_add(sqAF[:, 0], sqAF[:, 0], sqBF[:, 0])
            nc.scalar.activation(sqBF[:, 0], sqAF[:, 0], arsq, scale=16.0)
            nc.vector.tensor_mul(ea_lo, ea_lo, sqBF[:, 0])
            nc.vector.tensor_mul(ul, ul, sqBF[:, 0])
            # p0 sign slots + bin-256 patch
            nc.scalar.sign(ea_lo[0:1], ea_lo[0:1])         # sign(X0)
            nc.scalar.sign(ul[0:1], YaF[0:1, 1])           # sign(X512)
            nc.scalar.copy(YaF[0:1, 1], ebl[0:1])          # Ea256
            nc.scalar.activation(YbF[0:1, 1], vl[0:1],
                                 mybir.ActivationFunctionType.Copy,
                                 scale=-1.0)               # -Oa256
            # merged normalize of yc1/2/3 ([128, 6000] ops)
            YaA = flat2(Ya)
            YbA = flat2(Yb)
            sqAA, sqBA = flat2(sqA), flat2(sqB)
            nc.scalar.square(sqAA, YaA)
            nc.scalar.square(sqBA, YbA)
            nc.vector.tensor_add(sqAA, sqAA, sqBA)
            nc.scalar.activation(sqBA, sqAA, arsq, scale=16.0)
            nc.vector.tensor_mul(YaA, YaA, sqBA)
            nc.vector.tensor_mul(YbA, YbA, sqBA)
            nc.vector.tensor_add(flat2(Ys), YaA, YbA)
            nc.vector.tensor_sub(flat2(Yd), YaA, YbA)
            st["y0"] = (ea_lo, ul)

        def flat2(tl):
            return tl[:].rearrange("p c a b t -> p (c a b t)")

        blocks = []
        kb = 0
        for d in range(1, M):
            lanes = M - d
            for l0 in range(0, lanes, 2):
                lc = min(2, lanes - l0)
                blocks.append((d, l0, lc, kb))
            kb += lanes

        def emit_block(st, blk):
            b = st["b"]
            d, l0, lc, kb0 = blk
            rows = lc * T
            s1 = ds(l0 * T, rows)
            s2 = ds((l0 + d) * T, rows)
            ya0, yb0 = st["y0"]
            Ya, Yb, Ys, Yd = st["Y"]
            r_sb = r_pool.tile([128, 13, 2 * T], FP16, tag="r")
            nc.vector.tensor_mul(r_sb[:, 0, :rows], ya0[:, s1], ya0[:, s2])
            nc.vector.tensor_mul(r_sb[:, 1, :rows], yb0[:, s1], yb0[:, s2])
            nc.vector.tensor_mul(r_sb[:, 2, :rows], ya0[:, s1], yb0[:, s2])
            nc.vector.tensor_mul(r_sb[:, 3, :rows], yb0[:, s1], ya0[:, s2])
            # merged 3-chunk products: planes 4..6 k1, 7..9 k2, 10..12 k3
            def yck(tl, sl):
                return tl[:].rearrange("p c a b t -> p c (a b t)")[:, :, sl]
            nc.vector.tensor_mul(r_sb[:, 4:7, :rows], yck(Ys, s1), yck(Ya, s2))
            nc.vector.tensor_mul(r_sb[:, 7:10, :rows], yck(Ya, s1), yck(Ys, s2))
            nc.vector.tensor_mul(r_sb[:, 10:13, :rows], yck(Yb, s1), yck(Yd, s2))

            ps_o = inv_psum.tile([64, 2 * T], F32, tag="ops")
            for idx in range(13):
                nc.tensor.matmul(
                    ps_o[:, :rows],
                    g_sb[:, idx],
                    r_sb[:, idx, :rows],
                    start=(idx == 0), stop=(idx == 12),
                )
            o_sb = o_pool.tile([64, 2, T], F32, tag="osb")
            nc.scalar.copy(
                o_sb[:, :lc],
                ps_o[:, :rows].rearrange("p (l t) -> p l t", t=T),
            )
            nc.sync.dma_start(out[b, :, ds(kb0 + l0, lc)], o_sb[:, :lc])

        # software pipeline: batch b's pair blocks interleave batch b+1's
        # forward matmul steps so the PE never drains (p-state stays high)
        st = make_state(0)
        for s in STEP_ORDER:
            fwd_step(st, s)
        for b in range(NB):
            butterfly_norm(st)
            nxt = make_state(b + 1) if b + 1 < NB else None
            nxt_steps = list(STEP_ORDER)
            for i, blk in enumerate(blocks):
                emit_block(st, blk)
                if nxt is not None and i % 2 == 1 and nxt_steps:
                    fwd_step(nxt, nxt_steps.pop(0))
            if nxt is not None:
                while nxt_steps:
                    fwd_step(nxt, nxt_steps.pop(0))
            st = nxt
    nc.compile()
    return nc


_NC_CACHE = None


def kernel(x: np.ndarray) -> np.ndarray:
    global _NC_CACHE
    x = np.asarray(x, dtype=np.float32)
    assert x.shape == (B, M, T, L)
    perm = np.concatenate([np.arange(0, L, 2), np.arange(1, L, 2)])
    xT = np.ascontiguousarray(
        x.transpose(0, 1, 3, 2)[:, :, perm]).astype(np.float16)
    if _NC_CACHE is None:
        _NC_CACHE = build_bass()
    nc = _NC_CACHE
    in_maps = [{"xT": xT[c * NB:(c + 1) * NB]} for c in range(NCORES)]
    trace = bool(int(os.environ.get("GCC_TRACE", "0")))
    res = run_bass_kernel_spmd(nc, in_maps, core_ids=list(range(NCORES)),
                               trace=trace)
    if trace and res.exec_time_ns is not None:
        print(f"HW exec time: {res.exec_time_ns} ns")
        if res.instructions_and_trace is not None:
            print("trace:", res.instructions_and_trace[1])
    out = np.concatenate([r["out"] for r in res.results], axis=0)  # [B,NL,28diag,T]
    plist = [m * (2 * M - m - 1) // 2 + (m + d - m - 1)
             for d in range(1, M) for m in range(M - d)]
    final = np.empty((B, NPAIRS, T, NL), dtype=np.float32)
    final[:, plist] = out.transpose(0, 2, 3, 1)
    return final


# revision 25
# speedup vs baseline: 1.4046x; 1.0853x over previous
"""GCC-PHAT Trainium2 kernel (v3: radix-2 DIT forward, fp16 datapath).

Pipeline (per core, batch-sharded B=16 -> 2 per core):
  1. Host permutes samples even|odd: xT[b,m,j,t], j<512 = x[2j], j>=512 =
     x[2j+1]. Forward = two 512-point real DFTs (E over even, O over odd)
     sharing one F512 [512x512] fp16 stationary matrix; fp32 PSUM accum.
     F512 cols (4 chunks of 128): Ea[0..127], Ea[128..255],
     [Ea256, Eb 1..127], Eb[128..255]  (Eb = -sin coeffs).
  2. Radix-2 butterfly on DVE (tensor_scalar 4x + tensor_tensor 2x):
     u = c*oa + s*ob, v = c*ob - s*oa  (W^k = c - i s),
     X_lo = (ea+u, eb+v)  bins k / 128+k,
     X_hi = (ea-u, v-eb)  bins 512-k / 384-k (conjugate-reflected; the
     reversed bin order is baked into the inverse G rows).
     p0 lanes give bins 0 and 512 free; bin 256 patched from preserved
     eb_lo/ob_lo (= Ea256/Oa256).
  3. PHAT normalize per mic, all 8 mics per op ([128, 2000] tiles):
     squares on ACT, sum + y = X*w on DVE, w = Abs_reciprocal_sqrt(16 s)
     on ACT. y-chunk p0 specials: yc0 = (sign X0, sign X512).
  4. Pair products (28 pairs diag-major, 500-col blocks): 13 planes
     (yc0: aa/bb/ab/ba, yc1..3: Karatsuba k1/k2/k3), split DVE/Pool.
  5. Truncated inverse DFT: G [128x64] stationary, 13 accumulating
     matmuls per block into PSUM [64,500]; ACT copy -> DMA out
     [b, lag, diag-pair, t]; host unscrambles to [b, p, t, lag].
"""

import os
from contextlib import ExitStack

import numpy as np

import concourse.bass as bass
import concourse.bacc as bacc
import concourse.mybir as mybir
import concourse.tile as tile
from concourse.bass import ds, ts
from concourse.bass_utils import run_bass_kernel_spmd

B, M, T, L = 16, 8, 250, 1024
NCORES = 8
NB = B // NCORES
NPAIRS = (M * (M - 1)) // 2   # 28
NL = 64
F32 = mybir.dt.float32
FP16 = mybir.dt.float16


def _build_FEO() -> np.ndarray:
    """[2, 512, 512]: h=0 even-stream (n=2j), h=1 odd-stream (n=2j+1) DFT
    matrices with the radix-2 twiddle absorbed into the odd columns.
    Cols: 0..255 cos(2 pi n k / 1024) k=0..255; col 256 = cos(pi j)
    (bin 256 of the stream's own 512-DFT); 257..511 -sin(...) k=1..255."""
    j = np.arange(512, dtype=np.float64)[:, None]
    F = np.zeros((2, 512, 512))
    k_lo = np.arange(0, 128, dtype=np.float64)[None, :]
    k_hi = np.arange(128, 256, dtype=np.float64)[None, :]
    for h in range(2):
        n = 2 * j + h
        F[h, :, 0:128] = np.cos(2 * np.pi * n * k_lo / L)
        F[h, :, 128:256] = np.cos(2 * np.pi * n * k_hi / L)
        F[h, :, 256] = np.cos(np.pi * j[:, 0])
        F[h, :, 257:384] = -np.sin(2 * np.pi * n * k_lo[:, 1:] / L)
        F[h, :, 384:512] = -np.sin(2 * np.pi * n * k_hi / L)
    return F.astype(np.float16)


def _build_G13() -> np.ndarray:
    """[13, 128, NL]: planes 0..3 yc0 aa/bb/ab/ba; 4..6 yc1 k1/k2/k3;
    7..9 yc2; 10..12 yc3. k1=(a1+b1)a2 G=cos-sin; k2=a1(a2+b2) G=sin;
    k3=b1(a2-b2) G=-cos. Product scale 1/16 (y=unit/4) -> 16x in G,
    except yc0 p0 sign slots (scale 1)."""
    G = np.zeros((13, 128, NL))
    nj = (np.arange(NL) - 32).astype(np.float64)

    def cv(f):
        w = 1.0 if f == 512 else 2.0
        return 16.0 * w * np.cos(2 * np.pi * f * nj / L) / L

    def sv(f):
        w = 1.0 if f == 512 else 2.0
        return 16.0 * w * np.sin(2 * np.pi * f * nj / L) / L

    for p in range(1, 128):
        G[0, p] = cv(p)
        G[1, p] = cv(p)
        G[2, p] = sv(p)
        G[3, p] = -sv(p)
    G[0, 0] = 1.0 / L                   # bin 0 (sign products)
    G[1, 0] = np.cos(np.pi * nj) / L    # bin 512 (sign products)
    # planes 4..6 = k1 of (yc1, yc2, yc3); 7..9 = k2; 10..12 = k3 --
    # grouped by k so one merged DVE op writes 3 contiguous planes.
    for p in range(128):
        for j, f in ((0, 128 + p), (1, 256 if p == 0 else 512 - p),
                     (2, 384 - p)):
            G[4 + j, p] = cv(f) - sv(f)
            G[7 + j, p] = sv(f)
            G[10 + j, p] = -cv(f)
    return G.astype(np.float16)


def build_bass() -> bass.Bass:
    nc = bacc.Bacc("TRN2", target_bir_lowering=False, debug=False)
    xT = nc.dram_tensor("xT", [NB, M, L, T], FP16, kind="ExternalInput")
    out = nc.dram_tensor("out", [NB, NL, NPAIRS, T], F32, kind="ExternalOutput")
    Fh = nc.inline_tensor(np.ascontiguousarray(_build_FEO()), name="Fmat")
    Gh = nc.inline_tensor(np.ascontiguousarray(_build_G13()), name="Gmat")

    with tile.TileContext(nc) as tc, ExitStack() as ctx:
        consts = ctx.enter_context(tc.tile_pool(name="consts", bufs=1))
        xt_pool = ctx.enter_context(tc.tile_pool(name="xt", bufs=2))
        eo1_pool = ctx.enter_context(tc.tile_pool(name="eo1", bufs=1))
        eo2_pool = ctx.enter_context(tc.tile_pool(name="eo2", bufs=2))
        y2_pool = ctx.enter_context(tc.tile_pool(name="y2", bufs=2))
        y_pool = ctx.enter_context(tc.tile_pool(name="y", bufs=1))
        sq_pool = ctx.enter_context(tc.tile_pool(name="sq", bufs=1))
        r_pool = ctx.enter_context(tc.tile_pool(name="r", bufs=2))
        o_pool = ctx.enter_context(tc.tile_pool(name="o", bufs=2))
        fwd_psum = ctx.enter_context(tc.tile_pool(name="fps", bufs=6, space="PSUM"))
        inv_psum = ctx.enter_context(tc.tile_pool(name="ips", bufs=2, space="PSUM"))

        f_sb = consts.tile([128, 2, 4, 512], FP16)
        nc.sync.dma_start(f_sb[:], Fh[:].rearrange("h (k p) c -> p h k c", p=128))
        g_sb = consts.tile([128, 13, NL], FP16)
        nc.sync.dma_start(g_sb[:], Gh[:].rearrange("i p j -> p i j"))

        EO_NAMES = ["ea_lo", "ea_hi", "ebl", "eb_hi", "ul", "u_hi", "vl", "v_hi"]

        def flat(tl):
            return tl[:].rearrange("p a b t -> p (a b t)")

        def flat2(tl):
            return tl[:].rearrange("p c a b t -> p (c a b t)")

        def make_state(b):
            eo = {}
            for nm in EO_NAMES:
                # ea_lo/ul become yc0 (live through pair stage) -> 2 bufs
                pool = eo2_pool if nm in ("ea_lo", "ul") else eo1_pool
                eo[nm] = pool.tile([128, 4, 2, T], FP16, tag=nm, name=f"{nm}{b}")
            # Ya/Yb written by next batch's butterflies while this batch's
            # products still read them -> 2 bufs; Ys/Yd written in the gap.
            Ya = y2_pool.tile([128, 3, 4, 2, T], FP16, tag="Ya", name=f"Ya{b}")
            Yb = y2_pool.tile([128, 3, 4, 2, T], FP16, tag="Yb", name=f"Yb{b}")
            Ys = y_pool.tile([128, 3, 4, 2, T], FP16, tag="Ys", name=f"Ys{b}")
            Yd = y_pool.tile([128, 3, 4, 2, T], FP16, tag="Yd", name=f"Yd{b}")
            return dict(b=b, eo=eo, Y=(Ya, Yb, Ys, Yd))

        def fwd_step(st, mg):
            # one mic-group: all 8 (c, half) output chunks (32 matmuls)
            b = st["b"]
            xt_sb = xt_pool.tile([128, 8, 2, T], FP16, tag="xt")
            for mi in range(2):
                nc.sync.dma_start(
                    xt_sb[:, :, mi],
                    xT[b, 2 * mg + mi].rearrange("(k p) t -> p k t", p=128),
                )
            for c in range(4):
                for half in range(2):
                    dest = st["eo"][EO_NAMES[4 * half + c]]
                    ps = fwd_psum.tile([128, 2, T], F32, tag="fp")
                    for kc in range(4):
                        nc.tensor.matmul(
                            ps[:],
                            f_sb[:, half, kc, ts(c, 128)],
                            xt_sb[:, 4 * half + kc],
                            start=(kc == 0), stop=(kc == 3),
                        )
                    nc.scalar.copy(dest[:, mg], ps[:])

        def bn_stages(st):
            """Butterfly + normalize as a list of emission closures."""
            b = st["b"]
            eo = st["eo"]
            Ya, Yb, Ys, Yd = st["Y"]
            ea_lo, ea_hi = flat(eo["ea_lo"]), flat(eo["ea_hi"])
            ebl, eb_hi = flat(eo["ebl"]), flat(eo["eb_hi"])
            ul, u_hi = flat(eo["ul"]), flat(eo["u_hi"])
            vl, v_hi = flat(eo["vl"]), flat(eo["v_hi"])
            YaF = Ya[:].rearrange("p c a b t -> p c (a b t)")  # [128,3,2000]
            YbF = Yb[:].rearrange("p c a b t -> p c (a b t)")
            arsq = mybir.ActivationFunctionType.Abs_reciprocal_sqrt
            sqA = sq_pool.tile([128, 3, 4, 2, T], FP16, tag="sqA", name=f"sqA{b}")
            sqB = sq_pool.tile([128, 3, 4, 2, T], FP16, tag="sqB", name=f"sqB{b}")
            sqAF = sqA[:].rearrange("p c a b t -> p c (a b t)")
            sqBF = sqB[:].rearrange("p c a b t -> p c (a b t)")
            st["y0"] = (ea_lo, ul)

            def s1():  # butterflies lo-group (DVE)
                nc.vector.tensor_sub(YaF[:, 1], ea_lo, ul)     # yc2-a
                nc.vector.tensor_add(ea_lo, ea_lo, ul)         # yc0-a (in place)
                nc.vector.tensor_sub(YbF[:, 1], vl, ebl)       # yc2-b
                nc.vector.tensor_add(ul, ebl, vl)              # yc0-b -> ul

            def s2():  # butterflies hi-group
                nc.vector.tensor_sub(YaF[:, 2], ea_hi, u_hi)   # yc3-a
                nc.vector.tensor_add(YaF[:, 0], ea_hi, u_hi)   # yc1-a
                nc.vector.tensor_sub(YbF[:, 2], v_hi, eb_hi)   # yc3-b
                nc.vector.tensor_add(YbF[:, 0], eb_hi, v_hi)   # yc1-b

            def s3():  # yc0 squares
                nc.scalar.square(sqAF[:, 0], ea_lo)
                nc.scalar.square(sqBF[:, 0], ul)
                nc.vector.tensor_add(sqAF[:, 0], sqAF[:, 0], sqBF[:, 0])

            def s4():  # yc0 w + muls + sign slots + bin-256 patch
                nc.scalar.activation(sqBF[:, 0], sqAF[:, 0], arsq, scale=16.0)
                nc.vector.tensor_mul(ea_lo, ea_lo, sqBF[:, 0])
                nc.vector.tensor_mul(ul, ul, sqBF[:, 0])
                nc.scalar.sign(ea_lo[0:1], ea_lo[0:1])         # sign(X0)
                nc.scalar.sign(ul[0:1], YaF[0:1, 1])           # sign(X512)
                nc.scalar.copy(YaF[0:1, 1], ebl[0:1])          # Ea256
                nc.scalar.activation(YbF[0:1, 1], vl[0:1],
                                     mybir.ActivationFunctionType.Copy,
                                     scale=-1.0)               # -Oa256

            def s5():
                nc.scalar.square(flat2(sqA), flat2(Ya))

            def s6():
                nc.scalar.square(flat2(sqB), flat2(Yb))
                nc.vector.tensor_add(flat2(sqA), flat2(sqA), flat2(sqB))

            def s7():
                nc.scalar.activation(flat2(sqB), flat2(sqA), arsq, scale=16.0)
                nc.vector.tensor_mul(flat2(Ya), flat2(Ya), flat2(sqB))

            def s8():
                nc.vector.tensor_mul(flat2(Yb), flat2(Yb), flat2(sqB))

            def s9():  # Ys/Yd single-buffered: runs after prev batch's blocks
                nc.vector.tensor_add(flat2(Ys), flat2(Ya), flat2(Yb))
                nc.vector.tensor_sub(flat2(Yd), flat2(Ya), flat2(Yb))

            return [s1, s2, s3, s4, s5, s6, s7, s8, s9]

        blocks = []
        kb = 0
        for d in range(1, M):
            lanes = M - d
            for l0 in range(0, lanes, 2):
                lc = min(2, lanes - l0)
                blocks.append((d, l0, lc, kb))
            kb += lanes

        def emit_block(st, blk):
            b = st["b"]
            d, l0, lc, kb0 = blk
            rows = lc * T
            s1 = ds(l0 * T, rows)
            s2 = ds((l0 + d) * T, rows)
            ya0, yb0 = st["y0"]
            Ya, Yb, Ys, Yd = st["Y"]
            r_sb = r_pool.tile([128, 13, 2 * T], FP16, tag="r")
            nc.vector.tensor_mul(r_sb[:, 0, :rows], ya0[:, s1], ya0[:, s2])
            nc.vector.tensor_mul(r_sb[:, 1, :rows], yb0[:, s1], yb0[:, s2])
            nc.vector.tensor_mul(r_sb[:, 2, :rows], ya0[:, s1], yb0[:, s2])
            nc.vector.tensor_mul(r_sb[:, 3, :rows], yb0[:, s1], ya0[:, s2])
            # merged 3-chunk products: planes 4..6 k1, 7..9 k2, 10..12 k3
            def yck(tl, sl):
                return tl[:].rearrange("p c a b t -> p c (a b t)")[:, :, sl]
            nc.vector.tensor_mul(r_sb[:, 4:7, :rows], yck(Ys, s1), yck(Ya, s2))
            nc.vector.tensor_mul(r_sb[:, 7:10, :rows], yck(Ya, s1), yck(Ys, s2))
            nc.vector.tensor_mul(r_sb[:, 10:13, :rows], yck(Yb, s1), yck(Yd, s2))

            ps_o = inv_psum.tile([64, 2 * T], F32, tag="ops")
            for idx in range(13):
                nc.tensor.matmul(
                    ps_o[:, :rows],
                    g_sb[:, idx],
                    r_sb[:, idx, :rows],
                    start=(idx == 0), stop=(idx == 12),
                )
            o_sb = o_pool.tile([64, 2, T], F32, tag="osb")
            nc.scalar.copy(
                o_sb[:, :lc],
                ps_o[:, :rows].rearrange("p (l t) -> p l t", t=T),
            )
            nc.sync.dma_start(out[b, :, ds(kb0 + l0, lc)], o_sb[:, :lc])

        # software pipeline: batch b's pair blocks interleave batch b+1's
        # forward matmul steps (PE stays hot) and its butterfly/normalize
        # stages (kills the inter-batch trough).
        st = make_state(0)
        for mg in range(4):
            fwd_step(st, mg)
        for stage in bn_stages(st):
            stage()
        for b in range(NB):
            nxt = make_state(b + 1) if b + 1 < NB else None
            nxt_work = []
            if nxt is not None:
                nxt_stages = bn_stages(nxt)
                nxt_work = [lambda mg=mg: fwd_step(nxt, mg) for mg in range(4)]
                nxt_work += nxt_stages[:8]
                tail = nxt_stages[8:]
            for i, blk in enumerate(blocks):
                emit_block(st, blk)
                if nxt_work:
                    nxt_work.pop(0)()
            if nxt is not None:
                for w in nxt_work:
                    w()
                for stage in tail:
                    stage()
            st = nxt
    nc.compile()
    return nc


_NC_CACHE = None


def kernel(x: np.ndarray) -> np.ndarray:
    global _NC_CACHE
    x = np.asarray(x, dtype=np.float32)
    assert x.shape == (B, M, T, L)
    perm = np.concatenate([np.arange(0, L, 2), np.arange(1, L, 2)])
    xT = np.ascontiguousarray(
        x.transpose(0, 1, 3, 2)[:, :, perm]).astype(np.float16)
    if _NC_CACHE is None:
        _NC_CACHE = build_bass()
    nc = _NC_CACHE
    in_maps = [{"xT": xT[c * NB:(c + 1) * NB]} for c in range(NCORES)]
    trace = bool(int(os.environ.get("GCC_TRACE", "0")))
    res = run_bass_kernel_spmd(nc, in_maps, core_ids=list(range(NCORES)),
                               trace=trace)
    if trace and res.exec_time_ns is not None:
        print(f"HW exec time: {res.exec_time_ns} ns")
        if res.instructions_and_trace is not None:
            print("trace:", res.instructions_and_trace[1])
    out = np.concatenate([r["out"] for r in res.results], axis=0)  # [B,NL,28diag,T]
    plist = [m * (2 * M - m - 1) // 2 + (m + d - m - 1)
             for d in range(1, M) for m in range(M - d)]
    final = np.empty((B, NPAIRS, T, NL), dtype=np.float32)
    final[:, plist] = out.transpose(0, 2, 3, 1)
    return final


# revision 28
# speedup vs baseline: 1.4170x; 1.0088x over previous
"""GCC-PHAT Trainium2 kernel (v3: radix-2 DIT forward, fp16 datapath).

Pipeline (per core, batch-sharded B=16 -> 2 per core):
  1. Host permutes samples even|odd: xT[b,m,j,t], j<512 = x[2j], j>=512 =
     x[2j+1]. Forward = two 512-point real DFTs (E over even, O over odd)
     sharing one F512 [512x512] fp16 stationary matrix; fp32 PSUM accum.
     F512 cols (4 chunks of 128): Ea[0..127], Ea[128..255],
     [Ea256, Eb 1..127], Eb[128..255]  (Eb = -sin coeffs).
  2. Radix-2 butterfly on DVE (tensor_scalar 4x + tensor_tensor 2x):
     u = c*oa + s*ob, v = c*ob - s*oa  (W^k = c - i s),
     X_lo = (ea+u, eb+v)  bins k / 128+k,
     X_hi = (ea-u, v-eb)  bins 512-k / 384-k (conjugate-reflected; the
     reversed bin order is baked into the inverse G rows).
     p0 lanes give bins 0 and 512 free; bin 256 patched from preserved
     eb_lo/ob_lo (= Ea256/Oa256).
  3. PHAT normalize per mic, all 8 mics per op ([128, 2000] tiles):
     squares on ACT, sum + y = X*w on DVE, w = Abs_reciprocal_sqrt(16 s)
     on ACT. y-chunk p0 specials: yc0 = (sign X0, sign X512).
  4. Pair products (28 pairs diag-major, 500-col blocks): 13 planes
     (yc0: aa/bb/ab/ba, yc1..3: Karatsuba k1/k2/k3), split DVE/Pool.
  5. Truncated inverse DFT: G [128x64] stationary, 13 accumulating
     matmuls per block into PSUM [64,500]; ACT copy -> DMA out
     [b, lag, diag-pair, t]; host unscrambles to [b, p, t, lag].
"""

import os
from contextlib import ExitStack

import numpy as np

import concourse.bass as bass
import concourse.bacc as bacc
import concourse.mybir as mybir
import concourse.tile as tile
from concourse.bass import ds, ts
from concourse.bass_utils import run_bass_kernel_spmd

B, M, T, L = 16, 8, 250, 1024
NCORES = 8
NB = B // NCORES
NPAIRS = (M * (M - 1)) // 2   # 28
NL = 64
F32 = mybir.dt.float32
FP16 = mybir.dt.float16


def _build_FEO() -> np.ndarray:
    """[2, 512, 512]: h=0 even-stream (n=2j), h=1 odd-stream (n=2j+1) DFT
    matrices with the radix-2 twiddle absorbed into the odd columns.
    Cols: 0..255 cos(2 pi n k / 1024) k=0..255; col 256 = cos(pi j)
    (bin 256 of the stream's own 512-DFT); 257..511 -sin(...) k=1..255."""
    j = np.arange(512, dtype=np.float64)[:, None]
    F = np.zeros((2, 512, 512))
    k_lo = np.arange(0, 128, dtype=np.float64)[None, :]
    k_hi = np.arange(128, 256, dtype=np.float64)[None, :]
    for h in range(2):
        n = 2 * j + h
        F[h, :, 0:128] = np.cos(2 * np.pi * n * k_lo / L)
        F[h, :, 128:256] = np.cos(2 * np.pi * n * k_hi / L)
        F[h, :, 256] = np.cos(np.pi * j[:, 0])
        F[h, :, 257:384] = -np.sin(2 * np.pi * n * k_lo[:, 1:] / L)
        F[h, :, 384:512] = -np.sin(2 * np.pi * n * k_hi / L)
    return F.astype(np.float16)


def _build_G13() -> np.ndarray:
    """[13, 128, NL]: planes 0..3 yc0 aa/bb/ab/ba; 4..6 yc1 k1/k2/k3;
    7..9 yc2; 10..12 yc3. k1=(a1+b1)a2 G=cos-sin; k2=a1(a2+b2) G=sin;
    k3=b1(a2-b2) G=-cos. Product scale 1/16 (y=unit/4) -> 16x in G,
    except yc0 p0 sign slots (scale 1)."""
    G = np.zeros((13, 128, NL))
    nj = (np.arange(NL) - 32).astype(np.float64)

    def cv(f):
        w = 1.0 if f == 512 else 2.0
        return 16.0 * w * np.cos(2 * np.pi * f * nj / L) / L

    def sv(f):
        w = 1.0 if f == 512 else 2.0
        return 16.0 * w * np.sin(2 * np.pi * f * nj / L) / L

    for p in range(1, 128):
        G[0, p] = cv(p)
        G[1, p] = cv(p)
        G[2, p] = sv(p)
        G[3, p] = -sv(p)
    G[0, 0] = 1.0 / L                   # bin 0 (sign products)
    G[1, 0] = np.cos(np.pi * nj) / L    # bin 512 (sign products)
    # planes 4..6 = k1 of (yc1, yc2, yc3); 7..9 = k2; 10..12 = k3 --
    # grouped by k so one merged DVE op writes 3 contiguous planes.
    for p in range(128):
        for j, f in ((0, 128 + p), (1, 256 if p == 0 else 512 - p),
                     (2, 384 - p)):
            G[4 + j, p] = cv(f) - sv(f)
            G[7 + j, p] = sv(f)
            G[10 + j, p] = -cv(f)
    return G.astype(np.float16)


def build_bass() -> bass.Bass:
    nc = bacc.Bacc("TRN2", target_bir_lowering=False, debug=False)
    xT = nc.dram_tensor("xT", [NB, M, L, T], FP16, kind="ExternalInput")
    out = nc.dram_tensor("out", [NB, NL, NPAIRS, T], F32, kind="ExternalOutput")
    Fh = nc.inline_tensor(np.ascontiguousarray(_build_FEO()), name="Fmat")
    Gh = nc.inline_tensor(np.ascontiguousarray(_build_G13()), name="Gmat")

    with tile.TileContext(nc) as tc, ExitStack() as ctx:
        consts = ctx.enter_context(tc.tile_pool(name="consts", bufs=1))
        xt_pool = ctx.enter_context(tc.tile_pool(name="xt", bufs=2))
        eo1_pool = ctx.enter_context(tc.tile_pool(name="eo1", bufs=1))
        eo2_pool = ctx.enter_context(tc.tile_pool(name="eo2", bufs=2))
        y2_pool = ctx.enter_context(tc.tile_pool(name="y2", bufs=2))
        y_pool = ctx.enter_context(tc.tile_pool(name="y", bufs=1))
        sq_pool = ctx.enter_context(tc.tile_pool(name="sq", bufs=1))
        r_pool = ctx.enter_context(tc.tile_pool(name="r", bufs=2))
        o_pool = ctx.enter_context(tc.tile_pool(name="o", bufs=2))
        fwd_psum = ctx.enter_context(tc.tile_pool(name="fps", bufs=6, space="PSUM"))
        inv_psum = ctx.enter_context(tc.tile_pool(name="ips", bufs=2, space="PSUM"))

        f_sb = consts.tile([128, 2, 4, 512], FP16)
        nc.sync.dma_start(f_sb[:], Fh[:].rearrange("h (k p) c -> p h k c", p=128))
        g_sb = consts.tile([128, 13, NL], FP16)
        nc.sync.dma_start(g_sb[:], Gh[:].rearrange("i p j -> p i j"))

        EO_NAMES = ["ea_lo", "ea_hi", "ebl", "eb_hi", "ul", "u_hi", "vl", "v_hi"]

        def flat(tl):
            return tl[:].rearrange("p a b t -> p (a b t)")

        def flat2(tl):
            return tl[:].rearrange("p c a b t -> p (c a b t)")

        def make_state(b):
            eo = {}
            for nm in EO_NAMES:
                # ea_lo/ul become yc0 (live through pair stage) -> 2 bufs
                pool = eo2_pool if nm in ("ea_lo", "ul") else eo1_pool
                eo[nm] = pool.tile([128, 4, 2, T], FP16, tag=nm, name=f"{nm}{b}")
            # Ya/Yb written by next batch's butterflies while this batch's
            # products still read them -> 2 bufs; Ys/Yd written in the gap.
            Ya = y2_pool.tile([128, 3, 4, 2, T], FP16, tag="Ya", name=f"Ya{b}")
            Yb = y2_pool.tile([128, 3, 4, 2, T], FP16, tag="Yb", name=f"Yb{b}")
            Ys = y_pool.tile([128, 3, 4, 2, T], FP16, tag="Ys", name=f"Ys{b}")
            Yd = y_pool.tile([128, 3, 4, 2, T], FP16, tag="Yd", name=f"Yd{b}")
            return dict(b=b, eo=eo, Y=(Ya, Yb, Ys, Yd))

        def fwd_step(st, mg):
            # one mic-group: all 8 (c, half) output chunks (32 matmuls)
            b = st["b"]
            xt_sb = xt_pool.tile([128, 8, 2, T], FP16, tag="xt")
            for mi in range(2):
                nc.sync.dma_start(
                    xt_sb[:, :, mi],
                    xT[b, 2 * mg + mi].rearrange("(k p) t -> p k t", p=128),
                )
            for c in range(4):
                for half in range(2):
                    dest = st["eo"][EO_NAMES[4 * half + c]]
                    ps = fwd_psum.tile([128, 2, T], F32, tag="fp")
                    for kc in range(4):
                        nc.tensor.matmul(
                            ps[:],
                            f_sb[:, half, kc, ts(c, 128)],
                            xt_sb[:, 4 * half + kc],
                            start=(kc == 0), stop=(kc == 3),
                        )
                    nc.scalar.copy(dest[:, mg], ps[:])

        def bn_stages(st, dve_sq=False):
            """Butterfly + normalize as a list of emission closures.
            dve_sq: compute squares on DVE (batch 0: DVE is idle during
            startup while the ACT chain gates everything)."""
            b = st["b"]
            eo = st["eo"]
            Ya, Yb, Ys, Yd = st["Y"]
            ea_lo, ea_hi = flat(eo["ea_lo"]), flat(eo["ea_hi"])
            ebl, eb_hi = flat(eo["ebl"]), flat(eo["eb_hi"])
            ul, u_hi = flat(eo["ul"]), flat(eo["u_hi"])
            vl, v_hi = flat(eo["vl"]), flat(eo["v_hi"])
            YaF = Ya[:].rearrange("p c a b t -> p c (a b t)")  # [128,3,2000]
            YbF = Yb[:].rearrange("p c a b t -> p c (a b t)")
            arsq = mybir.ActivationFunctionType.Abs_reciprocal_sqrt
            sqA = sq_pool.tile([128, 3, 4, 2, T], FP16, tag="sqA", name=f"sqA{b}")
            sqB = sq_pool.tile([128, 3, 4, 2, T], FP16, tag="sqB", name=f"sqB{b}")
            sqAF = sqA[:].rearrange("p c a b t -> p c (a b t)")
            sqBF = sqB[:].rearrange("p c a b t -> p c (a b t)")
            st["y0"] = (ea_lo, ul)

            def s1():  # butterflies lo-group (DVE)
                nc.vector.tensor_sub(YaF[:, 1], ea_lo, ul)     # yc2-a
                nc.vector.tensor_add(ea_lo, ea_lo, ul)         # yc0-a (in place)
                nc.vector.tensor_sub(YbF[:, 1], vl, ebl)       # yc2-b
                nc.vector.tensor_add(ul, ebl, vl)              # yc0-b -> ul

            def s2():  # butterflies hi-group
                nc.vector.tensor_sub(YaF[:, 2], ea_hi, u_hi)   # yc3-a
                nc.vector.tensor_add(YaF[:, 0], ea_hi, u_hi)   # yc1-a
                nc.vector.tensor_sub(YbF[:, 2], v_hi, eb_hi)   # yc3-b
                nc.vector.tensor_add(YbF[:, 0], eb_hi, v_hi)   # yc1-b

            def sq(dst, src):
                if dve_sq:
                    nc.vector.tensor_mul(dst, src, src)
                else:
                    nc.scalar.square(dst, src)

            def s3():  # yc0 squares
                sq(sqAF[:, 0], ea_lo)
                sq(sqBF[:, 0], ul)
                nc.vector.tensor_add(sqAF[:, 0], sqAF[:, 0], sqBF[:, 0])

            def s4():  # yc0 w + muls + sign slots + bin-256 patch
                nc.scalar.activation(sqBF[:, 0], sqAF[:, 0], arsq, scale=16.0)
                nc.vector.tensor_mul(ea_lo, ea_lo, sqBF[:, 0])
                nc.vector.tensor_mul(ul, ul, sqBF[:, 0])
                nc.scalar.sign(ea_lo[0:1], ea_lo[0:1])         # sign(X0)
                nc.scalar.sign(ul[0:1], YaF[0:1, 1])           # sign(X512)
                nc.vector.tensor_scalar_mul(YaF[0:1, 1], ebl[0:1], 1.0)   # Ea256
                nc.vector.tensor_scalar_mul(YbF[0:1, 1], vl[0:1], -1.0)   # -Oa256

            def s5():
                sq(flat2(sqA), flat2(Ya))

            def s6():
                sq(flat2(sqB), flat2(Yb))
                nc.vector.tensor_add(flat2(sqA), flat2(sqA), flat2(sqB))

            def s7():
                nc.scalar.activation(flat2(sqB), flat2(sqA), arsq, scale=16.0)
                nc.vector.tensor_mul(flat2(Ya), flat2(Ya), flat2(sqB))

            def s8():
                nc.vector.tensor_mul(flat2(Yb), flat2(Yb), flat2(sqB))

            def s9():  # Ys/Yd single-buffered: runs after prev batch's blocks
                nc.vector.tensor_add(flat2(Ys), flat2(Ya), flat2(Yb))
                nc.vector.tensor_sub(flat2(Yd), flat2(Ya), flat2(Yb))

            return [s1, s2, s3, s4, s5, s6, s7, s8, s9]

        blocks = []
        kb = 0
        for d in range(1, M):
            lanes = M - d
            for l0 in range(0, lanes, 2):
                lc = min(2, lanes - l0)
                blocks.append((d, l0, lc, kb))
            kb += lanes

        def emit_block(st, blk):
            b = st["b"]
            d, l0, lc, kb0 = blk
            rows = lc * T
            s1 = ds(l0 * T, rows)
            s2 = ds((l0 + d) * T, rows)
            ya0, yb0 = st["y0"]
            Ya, Yb, Ys, Yd = st["Y"]
            r_sb = r_pool.tile([128, 13, 2 * T], FP16, tag="r")
            nc.vector.tensor_mul(r_sb[:, 0, :rows], ya0[:, s1], ya0[:, s2])
            nc.vector.tensor_mul(r_sb[:, 1, :rows], yb0[:, s1], yb0[:, s2])
            nc.vector.tensor_mul(r_sb[:, 2, :rows], ya0[:, s1], yb0[:, s2])
            nc.vector.tensor_mul(r_sb[:, 3, :rows], yb0[:, s1], ya0[:, s2])
            # merged 3-chunk products: planes 4..6 k1, 7..9 k2, 10..12 k3
            def yck(tl, sl):
                return tl[:].rearrange("p c a b t -> p c (a b t)")[:, :, sl]
            nc.vector.tensor_mul(r_sb[:, 4:7, :rows], yck(Ys, s1), yck(Ya, s2))
            nc.vector.tensor_mul(r_sb[:, 7:10, :rows], yck(Ya, s1), yck(Ys, s2))
            nc.vector.tensor_mul(r_sb[:, 10:13, :rows], yck(Yb, s1), yck(Yd, s2))

            ps_o = inv_psum.tile([64, 2 * T], F32, tag="ops")
            for idx in range(13):
                nc.tensor.matmul(
                    ps_o[:, :rows],
                    g_sb[:, idx],
                    r_sb[:, idx, :rows],
                    start=(idx == 0), stop=(idx == 12),
                )
            o_sb = o_pool.tile([64, 2, T], F32, tag="osb")
            nc.scalar.copy(
                o_sb[:, :lc],
                ps_o[:, :rows].rearrange("p (l t) -> p l t", t=T),
            )
            nc.sync.dma_start(out[b, :, ds(kb0 + l0, lc)], o_sb[:, :lc])

        # software pipeline: batch b's pair blocks interleave batch b+1's
        # forward matmul steps (PE stays hot) and its butterfly/normalize
        # stages (kills the inter-batch trough).
        st = make_state(0)
        for mg in range(4):
            fwd_step(st, mg)
        for stage in bn_stages(st, dve_sq=True):
            stage()
        for b in range(NB):
            nxt = make_state(b + 1) if b + 1 < NB else None
            nxt_work = []
            if nxt is not None:
                nxt_stages = bn_stages(nxt)
                nxt_work = [lambda mg=mg: fwd_step(nxt, mg) for mg in range(4)]
                nxt_work += nxt_stages[:8]
                tail = nxt_stages[8:]
            for i, blk in enumerate(blocks):
                emit_block(st, blk)
                if nxt_work:
                    nxt_work.pop(0)()
            if nxt is not None:
                for w in nxt_work:
                    w()
                for stage in tail:
                    stage()
            st = nxt
    nc.compile()
    return nc


_NC_CACHE = None


def kernel(x: np.ndarray) -> np.ndarray:
    global _NC_CACHE
    x = np.asarray(x, dtype=np.float32)
    assert x.shape == (B, M, T, L)
    perm = np.concatenate([np.arange(0, L, 2), np.arange(1, L, 2)])
    xT = np.ascontiguousarray(
        x.transpose(0, 1, 3, 2)[:, :, perm]).astype(np.float16)
    if _NC_CACHE is None:
        _NC_CACHE = build_bass()
    nc = _NC_CACHE
    in_maps = [{"xT": xT[c * NB:(c + 1) * NB]} for c in range(NCORES)]
    trace = bool(int(os.environ.get("GCC_TRACE", "0")))
    res = run_bass_kernel_spmd(nc, in_maps, core_ids=list(range(NCORES)),
                               trace=trace)
    if trace and res.exec_time_ns is not None:
        print(f"HW exec time: {res.exec_time_ns} ns")
        if res.instructions_and_trace is not None:
            print("trace:", res.instructions_and_trace[1])
    out = np.concatenate([r["out"] for r in res.results], axis=0)  # [B,NL,28diag,T]
    plist = [m * (2 * M - m - 1) // 2 + (m + d - m - 1)
             for d in range(1, M) for m in range(M - d)]
    final = np.empty((B, NPAIRS, T, NL), dtype=np.float32)
    final[:, plist] = out.transpose(0, 2, 3, 1)
    return final


# revision 34
# speedup vs baseline: 1.4582x; 1.0291x over previous
"""GCC-PHAT Trainium2 kernel (v3: radix-2 DIT forward, fp16 datapath).

Pipeline (per core, batch-sharded B=16 -> 2 per core):
  1. Host permutes samples even|odd: xT[b,m,j,t], j<512 = x[2j], j>=512 =
     x[2j+1]. Forward = two 512-point real DFTs (E over even, O over odd)
     sharing one F512 [512x512] fp16 stationary matrix; fp32 PSUM accum.
     F512 cols (4 chunks of 128): Ea[0..127], Ea[128..255],
     [Ea256, Eb 1..127], Eb[128..255]  (Eb = -sin coeffs).
  2. Radix-2 butterfly on DVE (tensor_scalar 4x + tensor_tensor 2x):
     u = c*oa + s*ob, v = c*ob - s*oa  (W^k = c - i s),
     X_lo = (ea+u, eb+v)  bins k / 128+k,
     X_hi = (ea-u, v-eb)  bins 512-k / 384-k (conjugate-reflected; the
     reversed bin order is baked into the inverse G rows).
     p0 lanes give bins 0 and 512 free; bin 256 patched from preserved
     eb_lo/ob_lo (= Ea256/Oa256).
  3. PHAT normalize per mic, all 8 mics per op ([128, 2000] tiles):
     squares on ACT, sum + y = X*w on DVE, w = Abs_reciprocal_sqrt(16 s)
     on ACT. y-chunk p0 specials: yc0 = (sign X0, sign X512).
  4. Pair products (28 pairs diag-major, 500-col blocks): 13 planes
     (yc0: aa/bb/ab/ba, yc1..3: Karatsuba k1/k2/k3), split DVE/Pool.
  5. Truncated inverse DFT: G [128x64] stationary, 13 accumulating
     matmuls per block into PSUM [64,500]; ACT copy -> DMA out
     [b, lag, diag-pair, t]; host unscrambles to [b, p, t, lag].
"""

import os
from contextlib import ExitStack

import numpy as np

import concourse.bass as bass
import concourse.bacc as bacc
import concourse.mybir as mybir
import concourse.tile as tile
from concourse.bass import ds, ts
from concourse.bass_utils import run_bass_kernel_spmd

B, M, T, L = 16, 8, 250, 1024
NCORES = 8
NB = B // NCORES
NPAIRS = (M * (M - 1)) // 2   # 28
NL = 64
F32 = mybir.dt.float32
FP16 = mybir.dt.float16


def _build_FEO() -> np.ndarray:
    """[2, 512, 512]: h=0 even-stream (n=2j), h=1 odd-stream (n=2j+1) DFT
    matrices with the radix-2 twiddle absorbed into the odd columns.
    Cols: 0..255 cos(2 pi n k / 1024) k=0..255; col 256 = cos(pi j)
    (bin 256 of the stream's own 512-DFT); 257..511 -sin(...) k=1..255."""
    j = np.arange(512, dtype=np.float64)[:, None]
    F = np.zeros((2, 512, 512))
    k_lo = np.arange(0, 128, dtype=np.float64)[None, :]
    k_hi = np.arange(128, 256, dtype=np.float64)[None, :]
    for h in range(2):
        n = 2 * j + h
        F[h, :, 0:128] = np.cos(2 * np.pi * n * k_lo / L)
        F[h, :, 128:256] = np.cos(2 * np.pi * n * k_hi / L)
        F[h, :, 256] = np.cos(np.pi * j[:, 0])
        F[h, :, 257:384] = -np.sin(2 * np.pi * n * k_lo[:, 1:] / L)
        F[h, :, 384:512] = -np.sin(2 * np.pi * n * k_hi / L)
    return F.astype(np.float16)


def _build_G13() -> np.ndarray:
    """[13, 128, NL]: planes 0..3 yc0 aa/bb/ab/ba; 4..6 yc1 k1/k2/k3;
    7..9 yc2; 10..12 yc3. k1=(a1+b1)a2 G=cos-sin; k2=a1(a2+b2) G=sin;
    k3=b1(a2-b2) G=-cos. Product scale 1/16 (y=unit/4) -> 16x in G,
    except yc0 p0 sign slots (scale 1)."""
    G = np.zeros((13, 128, NL))
    nj = (np.arange(NL) - 32).astype(np.float64)

    def cv(f):
        w = 1.0 if f == 512 else 2.0
        return 16.0 * w * np.cos(2 * np.pi * f * nj / L) / L

    def sv(f):
        w = 1.0 if f == 512 else 2.0
        return 16.0 * w * np.sin(2 * np.pi * f * nj / L) / L

    for p in range(1, 128):
        G[0, p] = cv(p)
        G[1, p] = cv(p)
        G[2, p] = sv(p)
        G[3, p] = -sv(p)
    G[0, 0] = 1.0 / L                   # bin 0 (sign products)
    G[1, 0] = np.cos(np.pi * nj) / L    # bin 512 (sign products)
    # planes 4..6 = k1 of (yc1, yc2, yc3); 7..9 = k2; 10..12 = k3 --
    # grouped by k so one merged DVE op writes 3 contiguous planes.
    for p in range(128):
        for j, f in ((0, 128 + p), (1, 256 if p == 0 else 512 - p),
                     (2, 384 - p)):
            G[4 + j, p] = cv(f) - sv(f)
            G[7 + j, p] = sv(f)
            G[10 + j, p] = -cv(f)
    return G.astype(np.float16)


def build_bass() -> bass.Bass:
    nc = bacc.Bacc("TRN2", target_bir_lowering=False, debug=False)
    xT = nc.dram_tensor("xT", [NB, M, L, T], FP16, kind="ExternalInput")
    out = nc.dram_tensor("out", [NB, NL, NPAIRS, T], F32, kind="ExternalOutput")
    Fh = nc.inline_tensor(np.ascontiguousarray(_build_FEO()), name="Fmat")
    Gh = nc.inline_tensor(np.ascontiguousarray(_build_G13()), name="Gmat")

    with tile.TileContext(nc) as tc, ExitStack() as ctx:
        consts = ctx.enter_context(tc.tile_pool(name="consts", bufs=1))
        xt_pool = ctx.enter_context(tc.tile_pool(name="xt", bufs=2))
        eo1_pool = ctx.enter_context(tc.tile_pool(name="eo1", bufs=1))
        eo2_pool = ctx.enter_context(tc.tile_pool(name="eo2", bufs=2))
        y2_pool = ctx.enter_context(tc.tile_pool(name="y2", bufs=2))
        y_pool = ctx.enter_context(tc.tile_pool(name="y", bufs=1))
        sq_pool = ctx.enter_context(tc.tile_pool(name="sq", bufs=1))
        r_pool = ctx.enter_context(tc.tile_pool(name="r", bufs=3))
        o_pool = ctx.enter_context(tc.tile_pool(name="o", bufs=2))
        fwd_psum = ctx.enter_context(tc.tile_pool(name="fps", bufs=6, space="PSUM"))
        inv_psum = ctx.enter_context(tc.tile_pool(name="ips", bufs=2, space="PSUM"))

        f_sb = consts.tile([128, 2, 4, 512], FP16)
        nc.sync.dma_start(f_sb[:], Fh[:].rearrange("h (k p) c -> p h k c", p=128))
        g_sb = consts.tile([128, 13, NL], FP16)
        nc.sync.dma_start(g_sb[:], Gh[:].rearrange("i p j -> p i j"))

        EO_NAMES = ["ea_lo", "ea_hi", "ebl", "eb_hi", "ul", "u_hi", "vl", "v_hi"]

        def flat(tl):
            return tl[:].rearrange("p a b t -> p (a b t)")

        def flat2(tl):
            return tl[:].rearrange("p c a b t -> p (c a b t)")

        def make_state(b):
            eo = {}
            for nm in EO_NAMES:
                # ea_lo/ul become yc0 (live through pair stage) -> 2 bufs
                pool = eo2_pool if nm in ("ea_lo", "ul") else eo1_pool
                eo[nm] = pool.tile([128, 4, 2, T], FP16, tag=nm, name=f"{nm}{b}")
            # Ya/Yb written by next batch's butterflies while this batch's
            # products still read them -> 2 bufs; Ys/Yd written in the gap.
            Ya = y2_pool.tile([128, 3, 4, 2, T], FP16, tag="Ya", name=f"Ya{b}")
            Yb = y2_pool.tile([128, 3, 4, 2, T], FP16, tag="Yb", name=f"Yb{b}")
            Ys = y_pool.tile([128, 3, 4, 2, T], FP16, tag="Ys", name=f"Ys{b}")
            Yd = y_pool.tile([128, 3, 4, 2, T], FP16, tag="Yd", name=f"Yd{b}")
            return dict(b=b, eo=eo, Y=(Ya, Yb, Ys, Yd))

        def fwd_step(st, mp):
            # one mic-group pair (mg = 2*mp, 2*mp+1): 64 matmuls
            b = st["b"]
            for mgi in range(2):
                mg = 2 * mp + mgi
                xt_sb = xt_pool.tile([128, 8, 2, T], FP16, tag="xt")
                for mi in range(2):
                    nc.sync.dma_start(
                        xt_sb[:, :, mi],
                        xT[b, 2 * mg + mi].rearrange("(k p) t -> p k t", p=128),
                    )
                for c in range(4):
                    for half in range(2):
                        dest = st["eo"][EO_NAMES[4 * half + c]]
                        ps = fwd_psum.tile([128, 2, T], F32, tag="fp")
                        for kc in range(4):
                            nc.tensor.matmul(
                                ps[:],
                                f_sb[:, half, kc, ts(c, 128)],
                                xt_sb[:, 4 * half + kc],
                                start=(kc == 0), stop=(kc == 3),
                            )
                        nc.scalar.copy(dest[:, mg], ps[:])

        def bn_stages(st, dve_sq=False):
            """Butterfly + normalize as a list of emission closures.
            dve_sq: compute squares on DVE (batch 0: DVE is idle during
            startup while the ACT chain gates everything)."""
            b = st["b"]
            eo = st["eo"]
            Ya, Yb, Ys, Yd = st["Y"]
            ea_lo, ea_hi = flat(eo["ea_lo"]), flat(eo["ea_hi"])
            ebl, eb_hi = flat(eo["ebl"]), flat(eo["eb_hi"])
            ul, u_hi = flat(eo["ul"]), flat(eo["u_hi"])
            vl, v_hi = flat(eo["vl"]), flat(eo["v_hi"])
            YaF = Ya[:].rearrange("p c a b t -> p c (a b t)")  # [128,3,2000]
            YbF = Yb[:].rearrange("p c a b t -> p c (a b t)")
            arsq = mybir.ActivationFunctionType.Abs_reciprocal_sqrt
            sqA = sq_pool.tile([128, 3, 4, 2, T], FP16, tag="sqA", name=f"sqA{b}")
            sqB = sq_pool.tile([128, 3, 4, 2, T], FP16, tag="sqB", name=f"sqB{b}")
            sqAF = sqA[:].rearrange("p c a b t -> p c (a b t)")
            sqBF = sqB[:].rearrange("p c a b t -> p c (a b t)")
            st["y0"] = (ea_lo, ul)

            def s1():  # butterflies lo-group (DVE)
                nc.vector.tensor_sub(YaF[:, 1], ea_lo, ul)     # yc2-a
                nc.vector.tensor_add(ea_lo, ea_lo, ul)         # yc0-a (in place)
                nc.vector.tensor_sub(YbF[:, 1], vl, ebl)       # yc2-b
                nc.vector.tensor_add(ul, ebl, vl)              # yc0-b -> ul

            def s2():  # butterflies hi-group
                nc.vector.tensor_sub(YaF[:, 2], ea_hi, u_hi)   # yc3-a
                nc.vector.tensor_add(YaF[:, 0], ea_hi, u_hi)   # yc1-a
                nc.vector.tensor_sub(YbF[:, 2], v_hi, eb_hi)   # yc3-b
                nc.vector.tensor_add(YbF[:, 0], eb_hi, v_hi)   # yc1-b

            def sq(dst, src):
                if dve_sq:
                    nc.vector.tensor_mul(dst, src, src)
                else:
                    nc.scalar.square(dst, src)

            def s3():  # yc0 squares
                sq(sqAF[:, 0], ea_lo)
                sq(sqBF[:, 0], ul)
                nc.vector.tensor_add(sqAF[:, 0], sqAF[:, 0], sqBF[:, 0])

            def s4():  # yc0 w + muls + sign slots + bin-256 patch
                nc.scalar.activation(sqBF[:, 0], sqAF[:, 0], arsq, scale=16.0)
                nc.vector.tensor_mul(ea_lo, ea_lo, sqBF[:, 0])
                nc.vector.tensor_mul(ul, ul, sqBF[:, 0])
                nc.scalar.sign(ea_lo[0:1], ea_lo[0:1])         # sign(X0)
                nc.scalar.sign(ul[0:1], YaF[0:1, 1])           # sign(X512)
                nc.vector.tensor_scalar_mul(YaF[0:1, 1], ebl[0:1], 1.0)   # Ea256
                nc.vector.tensor_scalar_mul(YbF[0:1, 1], vl[0:1], -1.0)   # -Oa256

            def hf(tl, h):
                return flat2(tl)[:, 3000 * h:3000 * (h + 1)]

            # big normalize in two halves so the ACT rsqrt of one half
            # overlaps the DVE muls of the other
            def s5():
                sq(hf(sqA, 0), hf(Ya, 0))
                sq(hf(sqB, 0), hf(Yb, 0))
                nc.vector.tensor_add(hf(sqA, 0), hf(sqA, 0), hf(sqB, 0))

            def s6():
                nc.scalar.activation(hf(sqB, 0), hf(sqA, 0), arsq, scale=16.0)
                sq(hf(sqA, 1), hf(Ya, 1))
                sq(hf(sqB, 1), hf(Yb, 1))
                nc.vector.tensor_add(hf(sqA, 1), hf(sqA, 1), hf(sqB, 1))

            def s7():
                nc.vector.tensor_mul(hf(Ya, 0), hf(Ya, 0), hf(sqB, 0))
                nc.vector.tensor_mul(hf(Yb, 0), hf(Yb, 0), hf(sqB, 0))
                nc.scalar.activation(hf(sqB, 1), hf(sqA, 1), arsq, scale=16.0)

            def s8():
                nc.vector.tensor_mul(hf(Ya, 1), hf(Ya, 1), hf(sqB, 1))
                nc.vector.tensor_mul(hf(Yb, 1), hf(Yb, 1), hf(sqB, 1))

            def s9():  # Ys/Yd single-buffered: runs after prev batch's blocks
                nc.vector.tensor_add(flat2(Ys), flat2(Ya), flat2(Yb))
                nc.vector.tensor_sub(flat2(Yd), flat2(Ya), flat2(Yb))

            return [s1, s2, s3, s4, s5, s6, s7, s8, s9]

        blocks = []
        kb = 0
        for d in range(1, M):
            lanes = M - d
            for l0 in range(0, lanes, 2):
                lc = min(2, lanes - l0)
                blocks.append((d, l0, lc, kb))
            kb += lanes

        def emit_block(st, blk):
            b = st["b"]
            d, l0, lc, kb0 = blk
            rows = lc * T
            s1 = ds(l0 * T, rows)
            s2 = ds((l0 + d) * T, rows)
            ya0, yb0 = st["y0"]
            Ya, Yb, Ys, Yd = st["Y"]
            r_sb = r_pool.tile([128, 13, 2 * T], FP16, tag="r")
            nc.vector.tensor_mul(r_sb[:, 0, :rows], ya0[:, s1], ya0[:, s2])
            nc.vector.tensor_mul(r_sb[:, 1, :rows], yb0[:, s1], yb0[:, s2])
            nc.vector.tensor_mul(r_sb[:, 2, :rows], ya0[:, s1], yb0[:, s2])
            nc.vector.tensor_mul(r_sb[:, 3, :rows], yb0[:, s1], ya0[:, s2])
            # merged 3-chunk products: planes 4..6 k1, 7..9 k2, 10..12 k3
            def yck(tl, sl):
                return tl[:].rearrange("p c a b t -> p c (a b t)")[:, :, sl]
            nc.vector.tensor_mul(r_sb[:, 4:7, :rows], yck(Ys, s1), yck(Ya, s2))
            nc.vector.tensor_mul(r_sb[:, 7:10, :rows], yck(Ya, s1), yck(Ys, s2))
            nc.vector.tensor_mul(r_sb[:, 10:13, :rows], yck(Yb, s1), yck(Yd, s2))

            ps_o = inv_psum.tile([64, 2 * T], F32, tag="ops")
            for idx in range(13):
                nc.tensor.matmul(
                    ps_o[:, :rows],
                    g_sb[:, idx],
                    r_sb[:, idx, :rows],
                    start=(idx == 0), stop=(idx == 12),
                )
            o_sb = o_pool.tile([64, 2, T], F32, tag="osb")
            nc.scalar.copy(
                o_sb[:, :lc],
                ps_o[:, :rows].rearrange("p (l t) -> p l t", t=T),
            )
            nc.sync.dma_start(out[b, :, ds(kb0 + l0, lc)], o_sb[:, :lc])

        # software pipeline: batch b's pair blocks interleave batch b+1's
        # forward matmul steps (PE stays hot) and its butterfly/normalize
        # stages (kills the inter-batch trough).
        st = make_state(0)
        for mp in range(2):
            fwd_step(st, mp)
        for stage in bn_stages(st, dve_sq=True):
            stage()
        for b in range(NB):
            nxt = make_state(b + 1) if b + 1 < NB else None
            nxt_work = []
            if nxt is not None:
                nxt_stages = bn_stages(nxt)
                nxt_work = [lambda mp=mp: fwd_step(nxt, mp) for mp in range(2)]
                nxt_work += nxt_stages[:8]
                tail = nxt_stages[8:]
            for i, blk in enumerate(blocks):
                emit_block(st, blk)
                if nxt_work:
                    nxt_work.pop(0)()
            if nxt is not None:
                for w in nxt_work:
                    w()
                for stage in tail:
                    stage()
            st = nxt
    nc.compile()
    return nc


_NC_CACHE = None


def kernel(x: np.ndarray) -> np.ndarray:
    global _NC_CACHE
    x = np.asarray(x, dtype=np.float32)
    assert x.shape == (B, M, T, L)
    perm = np.concatenate([np.arange(0, L, 2), np.arange(1, L, 2)])
    xT = np.ascontiguousarray(
        x.transpose(0, 1, 3, 2)[:, :, perm]).astype(np.float16)
    if _NC_CACHE is None:
        _NC_CACHE = build_bass()
    nc = _NC_CACHE
    in_maps = [{"xT": xT[c * NB:(c + 1) * NB]} for c in range(NCORES)]
    trace = bool(int(os.environ.get("GCC_TRACE", "0")))
    res = run_bass_kernel_spmd(nc, in_maps, core_ids=list(range(NCORES)),
                               trace=trace)
    if trace and res.exec_time_ns is not None:
        print(f"HW exec time: {res.exec_time_ns} ns")
        if res.instructions_and_trace is not None:
            print("trace:", res.instructions_and_trace[1])
    out = np.concatenate([r["out"] for r in res.results], axis=0)  # [B,NL,28diag,T]
    plist = [m * (2 * M - m - 1) // 2 + (m + d - m - 1)
             for d in range(1, M) for m in range(M - d)]
    final = np.empty((B, NPAIRS, T, NL), dtype=np.float32)
    final[:, plist] = out.transpose(0, 2, 3, 1)
    return final


# revision 36
# speedup vs baseline: 1.4595x; 1.0009x over previous
"""GCC-PHAT Trainium2 kernel (v3: radix-2 DIT forward, fp16 datapath).

Pipeline (per core, batch-sharded B=16 -> 2 per core):
  1. Host permutes samples even|odd: xT[b,m,j,t], j<512 = x[2j], j>=512 =
     x[2j+1]. Forward = two 512-point real DFTs (E over even, O over odd)
     sharing one F512 [512x512] fp16 stationary matrix; fp32 PSUM accum.
     F512 cols (4 chunks of 128): Ea[0..127], Ea[128..255],
     [Ea256, Eb 1..127], Eb[128..255]  (Eb = -sin coeffs).
  2. Radix-2 butterfly on DVE (tensor_scalar 4x + tensor_tensor 2x):
     u = c*oa + s*ob, v = c*ob - s*oa  (W^k = c - i s),
     X_lo = (ea+u, eb+v)  bins k / 128+k,
     X_hi = (ea-u, v-eb)  bins 512-k / 384-k (conjugate-reflected; the
     reversed bin order is baked into the inverse G rows).
     p0 lanes give bins 0 and 512 free; bin 256 patched from preserved
     eb_lo/ob_lo (= Ea256/Oa256).
  3. PHAT normalize per mic, all 8 mics per op ([128, 2000] tiles):
     squares on ACT, sum + y = X*w on DVE, w = Abs_reciprocal_sqrt(16 s)
     on ACT. y-chunk p0 specials: yc0 = (sign X0, sign X512).
  4. Pair products (28 pairs diag-major, 500-col blocks): 13 planes
     (yc0: aa/bb/ab/ba, yc1..3: Karatsuba k1/k2/k3), split DVE/Pool.
  5. Truncated inverse DFT: G [128x64] stationary, 13 accumulating
     matmuls per block into PSUM [64,500]; ACT copy -> DMA out
     [b, lag, diag-pair, t]; host unscrambles to [b, p, t, lag].
"""

import os
from contextlib import ExitStack

import numpy as np

import concourse.bass as bass
import concourse.bacc as bacc
import concourse.mybir as mybir
import concourse.tile as tile
from concourse.bass import ds, ts
from concourse.bass_utils import run_bass_kernel_spmd

B, M, T, L = 16, 8, 250, 1024
NCORES = 8
NB = B // NCORES
NPAIRS = (M * (M - 1)) // 2   # 28
NL = 64
F32 = mybir.dt.float32
FP16 = mybir.dt.float16


def _build_FEO() -> np.ndarray:
    """[2, 512, 512]: h=0 even-stream (n=2j), h=1 odd-stream (n=2j+1) DFT
    matrices with the radix-2 twiddle absorbed into the odd columns.
    Cols: 0..255 cos(2 pi n k / 1024) k=0..255; col 256 = cos(pi j)
    (bin 256 of the stream's own 512-DFT); 257..511 -sin(...) k=1..255."""
    j = np.arange(512, dtype=np.float64)[:, None]
    F = np.zeros((2, 512, 512))
    k_lo = np.arange(0, 128, dtype=np.float64)[None, :]
    k_hi = np.arange(128, 256, dtype=np.float64)[None, :]
    for h in range(2):
        n = 2 * j + h
        F[h, :, 0:128] = np.cos(2 * np.pi * n * k_lo / L)
        F[h, :, 128:256] = np.cos(2 * np.pi * n * k_hi / L)
        F[h, :, 256] = np.cos(np.pi * j[:, 0])
        F[h, :, 257:384] = -np.sin(2 * np.pi * n * k_lo[:, 1:] / L)
        F[h, :, 384:512] = -np.sin(2 * np.pi * n * k_hi / L)
    return F.astype(np.float16)


def _build_G13() -> np.ndarray:
    """[13, 128, NL]: planes 0..3 yc0 aa/bb/ab/ba; 4..6 yc1 k1/k2/k3;
    7..9 yc2; 10..12 yc3. k1=(a1+b1)a2 G=cos-sin; k2=a1(a2+b2) G=sin;
    k3=b1(a2-b2) G=-cos. Product scale 1/16 (y=unit/4) -> 16x in G,
    except yc0 p0 sign slots (scale 1)."""
    G = np.zeros((13, 128, NL))
    nj = (np.arange(NL) - 32).astype(np.float64)

    def cv(f):
        w = 1.0 if f == 512 else 2.0
        return 16.0 * w * np.cos(2 * np.pi * f * nj / L) / L

    def sv(f):
        w = 1.0 if f == 512 else 2.0
        return 16.0 * w * np.sin(2 * np.pi * f * nj / L) / L

    for p in range(1, 128):
        G[0, p] = cv(p)
        G[1, p] = cv(p)
        G[2, p] = sv(p)
        G[3, p] = -sv(p)
    G[0, 0] = 1.0 / L                   # bin 0 (sign products)
    G[1, 0] = np.cos(np.pi * nj) / L    # bin 512 (sign products)
    # planes 4..6 = k1 of (yc1, yc2, yc3); 7..9 = k2; 10..12 = k3 --
    # grouped by k so one merged DVE op writes 3 contiguous planes.
    for p in range(128):
        for j, f in ((0, 128 + p), (1, 256 if p == 0 else 512 - p),
                     (2, 384 - p)):
            G[4 + j, p] = cv(f) - sv(f)
            G[7 + j, p] = sv(f)
            G[10 + j, p] = -cv(f)
    return G.astype(np.float16)


def build_bass() -> bass.Bass:
    nc = bacc.Bacc("TRN2", target_bir_lowering=False, debug=False)
    xT = nc.dram_tensor("xT", [NB, M, L, T], FP16, kind="ExternalInput")
    out = nc.dram_tensor("out", [NB, NL, NPAIRS, T], F32, kind="ExternalOutput")
    Fh = nc.inline_tensor(np.ascontiguousarray(_build_FEO()), name="Fmat")
    Gh = nc.inline_tensor(np.ascontiguousarray(_build_G13()), name="Gmat")

    with tile.TileContext(nc) as tc, ExitStack() as ctx:
        consts = ctx.enter_context(tc.tile_pool(name="consts", bufs=1))
        xt_pool = ctx.enter_context(tc.tile_pool(name="xt", bufs=2))
        eo1_pool = ctx.enter_context(tc.tile_pool(name="eo1", bufs=1))
        eo2_pool = ctx.enter_context(tc.tile_pool(name="eo2", bufs=2))
        y2_pool = ctx.enter_context(tc.tile_pool(name="y2", bufs=2))
        y_pool = ctx.enter_context(tc.tile_pool(name="y", bufs=1))
        sq_pool = ctx.enter_context(tc.tile_pool(name="sq", bufs=1))
        r_pool = ctx.enter_context(tc.tile_pool(name="r", bufs=3))
        o_pool = ctx.enter_context(tc.tile_pool(name="o", bufs=2))
        fwd_psum = ctx.enter_context(tc.tile_pool(name="fps", bufs=6, space="PSUM"))
        inv_psum = ctx.enter_context(tc.tile_pool(name="ips", bufs=2, space="PSUM"))

        f_sb = consts.tile([128, 2, 4, 512], FP16)
        nc.sync.dma_start(f_sb[:], Fh[:].rearrange("h (k p) c -> p h k c", p=128))
        g_sb = consts.tile([128, 13, NL], FP16)
        nc.sync.dma_start(g_sb[:], Gh[:].rearrange("i p j -> p i j"))

        EO_NAMES = ["ea_lo", "ea_hi", "ebl", "eb_hi", "ul", "u_hi", "vl", "v_hi"]

        def flat(tl):
            return tl[:].rearrange("p a b t -> p (a b t)")

        def flat2(tl):
            return tl[:].rearrange("p c a b t -> p (c a b t)")

        def make_state(b):
            eo = {}
            for nm in EO_NAMES:
                # ea_lo/ul become yc0 (live through pair stage) -> 2 bufs
                pool = eo2_pool if nm in ("ea_lo", "ul") else eo1_pool
                eo[nm] = pool.tile([128, 4, 2, T], FP16, tag=nm, name=f"{nm}{b}")
            # Ya/Yb written by next batch's butterflies while this batch's
            # products still read them -> 2 bufs; Ys/Yd written in the gap.
            Ya = y2_pool.tile([128, 3, 4, 2, T], FP16, tag="Ya", name=f"Ya{b}")
            Yb = y2_pool.tile([128, 3, 4, 2, T], FP16, tag="Yb", name=f"Yb{b}")
            Ys = y_pool.tile([128, 3, 4, 2, T], FP16, tag="Ys", name=f"Ys{b}")
            Yd = y_pool.tile([128, 3, 4, 2, T], FP16, tag="Yd", name=f"Yd{b}")
            return dict(b=b, eo=eo, Y=(Ya, Yb, Ys, Yd))

        def fwd_step(st, mp, dve_drain=False):
            # one mic-group pair (mg = 2*mp, 2*mp+1): 64 matmuls.
            # dve_drain (batch 0): drain the butterfly-gating chunks (c0/c2)
            # on the then-idle DVE to halve the ACT drain-queue latency.
            b = st["b"]
            for mgi in range(2):
                mg = 2 * mp + mgi
                xt_sb = xt_pool.tile([128, 8, 2, T], FP16, tag="xt")
                for mi in range(2):
                    nc.sync.dma_start(
                        xt_sb[:, :, mi],
                        xT[b, 2 * mg + mi].rearrange("(k p) t -> p k t", p=128),
                    )
                for c in range(4):
                    for half in range(2):
                        dest = st["eo"][EO_NAMES[4 * half + c]]
                        ps = fwd_psum.tile([128, 2, T], F32, tag="fp")
                        for kc in range(4):
                            nc.tensor.matmul(
                                ps[:],
                                f_sb[:, half, kc, ts(c, 128)],
                                xt_sb[:, 4 * half + kc],
                                start=(kc == 0), stop=(kc == 3),
                            )
                        if dve_drain and c in (0, 2):
                            nc.vector.tensor_scalar_mul(dest[:, mg], ps[:], 1.0)
                        else:
                            nc.scalar.copy(dest[:, mg], ps[:])

        def bn_stages(st, dve_sq=False):
            """Butterfly + normalize as a list of emission closures.
            dve_sq: compute squares on DVE (batch 0: DVE is idle during
            startup while the ACT chain gates everything)."""
            b = st["b"]
            eo = st["eo"]
            Ya, Yb, Ys, Yd = st["Y"]
            ea_lo, ea_hi = flat(eo["ea_lo"]), flat(eo["ea_hi"])
            ebl, eb_hi = flat(eo["ebl"]), flat(eo["eb_hi"])
            ul, u_hi = flat(eo["ul"]), flat(eo["u_hi"])
            vl, v_hi = flat(eo["vl"]), flat(eo["v_hi"])
            YaF = Ya[:].rearrange("p c a b t -> p c (a b t)")  # [128,3,2000]
            YbF = Yb[:].rearrange("p c a b t -> p c (a b t)")
            arsq = mybir.ActivationFunctionType.Abs_reciprocal_sqrt
            sqA = sq_pool.tile([128, 3, 4, 2, T], FP16, tag="sqA", name=f"sqA{b}")
            sqB = sq_pool.tile([128, 3, 4, 2, T], FP16, tag="sqB", name=f"sqB{b}")
            sqAF = sqA[:].rearrange("p c a b t -> p c (a b t)")
            sqBF = sqB[:].rearrange("p c a b t -> p c (a b t)")
            st["y0"] = (ea_lo, ul)

            def s1():  # butterflies lo-group (DVE)
                nc.vector.tensor_sub(YaF[:, 1], ea_lo, ul)     # yc2-a
                nc.vector.tensor_add(ea_lo, ea_lo, ul)         # yc0-a (in place)
                nc.vector.tensor_sub(YbF[:, 1], vl, ebl)       # yc2-b
                nc.vector.tensor_add(ul, ebl, vl)              # yc0-b -> ul

            def s2():  # butterflies hi-group
                nc.vector.tensor_sub(YaF[:, 2], ea_hi, u_hi)   # yc3-a
                nc.vector.tensor_add(YaF[:, 0], ea_hi, u_hi)   # yc1-a
                nc.vector.tensor_sub(YbF[:, 2], v_hi, eb_hi)   # yc3-b
                nc.vector.tensor_add(YbF[:, 0], eb_hi, v_hi)   # yc1-b

            def sq(dst, src):
                if dve_sq:
                    nc.vector.tensor_mul(dst, src, src)
                else:
                    nc.scalar.square(dst, src)

            def s3():  # yc0 squares
                sq(sqAF[:, 0], ea_lo)
                sq(sqBF[:, 0], ul)
                nc.vector.tensor_add(sqAF[:, 0], sqAF[:, 0], sqBF[:, 0])

            def s4():  # yc0 w + muls + sign slots + bin-256 patch
                nc.scalar.activation(sqBF[:, 0], sqAF[:, 0], arsq, scale=16.0)
                nc.vector.tensor_mul(ea_lo, ea_lo, sqBF[:, 0])
                nc.vector.tensor_mul(ul, ul, sqBF[:, 0])
                nc.scalar.sign(ea_lo[0:1], ea_lo[0:1])         # sign(X0)
                nc.scalar.sign(ul[0:1], YaF[0:1, 1])           # sign(X512)
                nc.vector.tensor_scalar_mul(YaF[0:1, 1], ebl[0:1], 1.0)   # Ea256
                nc.vector.tensor_scalar_mul(YbF[0:1, 1], vl[0:1], -1.0)   # -Oa256

            def hf(tl, h):
                return flat2(tl)[:, 3000 * h:3000 * (h + 1)]

            # big normalize in two halves so the ACT rsqrt of one half
            # overlaps the DVE muls of the other
            def s5():
                sq(hf(sqA, 0), hf(Ya, 0))
                sq(hf(sqB, 0), hf(Yb, 0))
                nc.vector.tensor_add(hf(sqA, 0), hf(sqA, 0), hf(sqB, 0))

            def s6():
                nc.scalar.activation(hf(sqB, 0), hf(sqA, 0), arsq, scale=16.0)
                sq(hf(sqA, 1), hf(Ya, 1))
                sq(hf(sqB, 1), hf(Yb, 1))
                nc.vector.tensor_add(hf(sqA, 1), hf(sqA, 1), hf(sqB, 1))

            def s7():
                nc.vector.tensor_mul(hf(Ya, 0), hf(Ya, 0), hf(sqB, 0))
                nc.vector.tensor_mul(hf(Yb, 0), hf(Yb, 0), hf(sqB, 0))
                nc.scalar.activation(hf(sqB, 1), hf(sqA, 1), arsq, scale=16.0)

            def s8():
                nc.vector.tensor_mul(hf(Ya, 1), hf(Ya, 1), hf(sqB, 1))
                nc.vector.tensor_mul(hf(Yb, 1), hf(Yb, 1), hf(sqB, 1))

            def s9():  # Ys/Yd single-buffered: runs after prev batch's blocks
                nc.vector.tensor_add(flat2(Ys), flat2(Ya), flat2(Yb))
                nc.vector.tensor_sub(flat2(Yd), flat2(Ya), flat2(Yb))

            return [s1, s2, s3, s4, s5, s6, s7, s8, s9]

        blocks = []
        kb = 0
        for d in range(1, M):
            lanes = M - d
            for l0 in range(0, lanes, 2):
                lc = min(2, lanes - l0)
                blocks.append((d, l0, lc, kb))
            kb += lanes

        def emit_block(st, blk):
            b = st["b"]
            d, l0, lc, kb0 = blk
            rows = lc * T
            s1 = ds(l0 * T, rows)
            s2 = ds((l0 + d) * T, rows)
            ya0, yb0 = st["y0"]
            Ya, Yb, Ys, Yd = st["Y"]
            r_sb = r_pool.tile([128, 13, 2 * T], FP16, tag="r")
            nc.vector.tensor_mul(r_sb[:, 0, :rows], ya0[:, s1], ya0[:, s2])
            nc.vector.tensor_mul(r_sb[:, 1, :rows], yb0[:, s1], yb0[:, s2])
            nc.vector.tensor_mul(r_sb[:, 2, :rows], ya0[:, s1], yb0[:, s2])
            nc.vector.tensor_mul(r_sb[:, 3, :rows], yb0[:, s1], ya0[:, s2])
            # merged 3-chunk products: planes 4..6 k1, 7..9 k2, 10..12 k3
            def yck(tl, sl):
                return tl[:].rearrange("p c a b t -> p c (a b t)")[:, :, sl]
            nc.vector.tensor_mul(r_sb[:, 4:7, :rows], yck(Ys, s1), yck(Ya, s2))
            nc.vector.tensor_mul(r_sb[:, 7:10, :rows], yck(Ya, s1), yck(Ys, s2))
            nc.vector.tensor_mul(r_sb[:, 10:13, :rows], yck(Yb, s1), yck(Yd, s2))

            ps_o = inv_psum.tile([64, 2 * T], F32, tag="ops")
            for idx in range(13):
                nc.tensor.matmul(
                    ps_o[:, :rows],
                    g_sb[:, idx],
                    r_sb[:, idx, :rows],
                    start=(idx == 0), stop=(idx == 12),
                )
            o_sb = o_pool.tile([64, 2, T], F32, tag="osb")
            nc.scalar.copy(
                o_sb[:, :lc],
                ps_o[:, :rows].rearrange("p (l t) -> p l t", t=T),
            )
            nc.sync.dma_start(out[b, :, ds(kb0 + l0, lc)], o_sb[:, :lc])

        # software pipeline: batch b's pair blocks interleave batch b+1's
        # forward matmul steps (PE stays hot) and its butterfly/normalize
        # stages (kills the inter-batch trough).
        st = make_state(0)
        for mp in range(2):
            fwd_step(st, mp, dve_drain=True)
        for stage in bn_stages(st, dve_sq=True):
            stage()
        for b in range(NB):
            nxt = make_state(b + 1) if b + 1 < NB else None
            nxt_work = []
            if nxt is not None:
                nxt_stages = bn_stages(nxt)
                nxt_work = [lambda mp=mp: fwd_step(nxt, mp) for mp in range(2)]
                nxt_work += nxt_stages[:8]
                tail = nxt_stages[8:]
            for i, blk in enumerate(blocks):
                emit_block(st, blk)
                if nxt_work:
                    nxt_work.pop(0)()
            if nxt is not None:
                for w in nxt_work:
                    w()
                for stage in tail:
                    stage()
            st = nxt
    nc.compile()
    return nc


_NC_CACHE = None


def kernel(x: np.ndarray) -> np.ndarray:
    global _NC_CACHE
    x = np.asarray(x, dtype=np.float32)
    assert x.shape == (B, M, T, L)
    perm = np.concatenate([np.arange(0, L, 2), np.arange(1, L, 2)])
    xT = np.ascontiguousarray(
        x.transpose(0, 1, 3, 2)[:, :, perm]).astype(np.float16)
    if _NC_CACHE is None:
        _NC_CACHE = build_bass()
    nc = _NC_CACHE
    in_maps = [{"xT": xT[c * NB:(c + 1) * NB]} for c in range(NCORES)]
    trace = bool(int(os.environ.get("GCC_TRACE", "0")))
    res = run_bass_kernel_spmd(nc, in_maps, core_ids=list(range(NCORES)),
                               trace=trace)
    if trace and res.exec_time_ns is not None:
        print(f"HW exec time: {res.exec_time_ns} ns")
        if res.instructions_and_trace is not None:
            print("trace:", res.instructions_and_trace[1])
    out = np.concatenate([r["out"] for r in res.results], axis=0)  # [B,NL,28diag,T]
    plist = [m * (2 * M - m - 1) // 2 + (m + d - m - 1)
             for d in range(1, M) for m in range(M - d)]
    final = np.empty((B, NPAIRS, T, NL), dtype=np.float32)
    final[:, plist] = out.transpose(0, 2, 3, 1)
    return final
